# revision 27
# baseline (speedup 1.0000x reference)
"""Multi-head causal attention (B=2, T=2048, C=1024, H=16) on 8 trn2 cores.

Sharding: tensor-parallel over heads. Each core computes 2 heads' QKV
projections + attention + a partial output projection; the host sums the
8 partial projections and adds the output bias.

v2: pipelined emission (QKV-projection groups interleaved with attention
i-tiles so the PE never drains), per-i-tile softmax normalization via
reciprocal_approx_fast + a K=2 broadcast matmul (replaces the serial
[1,2048] DVE reciprocal that idled the PE past the HAM window), 2-head
score matmuls packed into one PE slot via row tiling, exp merged over
both heads' PSUM banks, mask-muls on the idle GpSimd engine, bf16
partial outputs.
"""

import contextlib
import os

import ml_dtypes
import numpy as np

import bass_rust
import concourse.bass as bass
import concourse.mybir as mybir
import concourse.tile as tile
from concourse.bass_utils import run_bass_kernel_spmd

F32 = mybir.dt.float32
F32R = mybir.dt.float32r
BF16 = mybir.dt.bfloat16
NPBF16 = ml_dtypes.bfloat16

B, T, C, H = 2, 2048, 1024, 16
D = C // H          # 64
NCORES = 8
HL = H // NCORES    # heads per core = 2
TOK = B * T         # 4096
HC = HL * D         # local head channels = 128

NT = TOK // 512     # 8 token column tiles (512) over both batches
KT = C // 128       # 8 contraction tiles for projections
QT = T // 512       # 4 q tiles per batch
JB = T // 128       # 16 j (key) blocks per batch

_MAXW = 1


def _patched_drain_and_barrier(self, tick_clock, wait_clock):
    """Stock tile tail drain carries one sem-wait per outstanding proc on a
    single TPB_CTRL drain; this walrus build allows only one sync-wait per
    ctrl instruction. Split the waits across no-op carriers."""
    nc = self.nc
    carrier = nc.sync.nop()
    wait_clock.add_sem_waits(
        carrier.ins, bass_rust.ScopedClock({None: tick_clock.global_clock})
    )
    si = carrier.ins.sync_info
    waits = list(si.on_wait) if si and si.on_wait else []
    if len(waits) > _MAXW:
        carrier.ins.sync_info = mybir.SyncInfo(
            on_wait=waits[:_MAXW], on_update=list(si.on_update or [])
        )
        for i in range(_MAXW, len(waits), _MAXW):
            nop = nc.sync.nop()
            nop.ins.sync_info = mybir.SyncInfo(
                on_wait=waits[i : i + _MAXW], on_update=[]
            )
    nc.sync.drain()

    nc.all_engine_barrier()
    popped = nc._tile_sem_poison_stack.pop()
    assert popped is self._sem_poison
    assert self.sems is not None
    nc.clear_and_free_semaphores(list(self.sems.allocated().values()))
    nc.all_engine_barrier()


tile.TileContext._drain_and_barrier = _patched_drain_and_barrier


def _split_waits(nc, maxw=_MAXW):
    """This walrus build accepts at most one sync-wait per instruction.
    Hoist excess waits onto no-op carriers inserted just before the
    instruction on the same engine."""
    for f in nc.m.functions:
        for bb in f.blocks:
            insts = bb.instructions
            if not any(
                i.sync_info and i.sync_info.on_wait and len(i.sync_info.on_wait) > maxw
                for i in insts
            ):
                continue
            new = []
            for inst in insts:
                si = inst.sync_info
                waits = list(si.on_wait) if si and si.on_wait else []
                if len(waits) > maxw:
                    keep = waits[-maxw:]
                    extra = waits[:-maxw]
                    for j in range(0, len(extra), maxw):
                        nop = mybir.InstNoOp(name=nc.get_next_instruction_name())
                        nop.engine = inst.engine
                        nop.sync_info = mybir.SyncInfo(
                            on_wait=extra[j : j + maxw], on_update=[]
                        )
                        nc.register_instruction(nop)
                        new.append(nop)
                    inst.sync_info = mybir.SyncInfo(
                        on_wait=keep, on_update=list(si.on_update or [])
                    )
                new.append(inst)
            bb.instructions = new


def build(with_bias):
    nc = bass.Bass()
    # x3[p, a, m] = x.T[a*128 + p, m] — pre-rearranged on host so one DMA
    # fetches a [128, 8, 512] contraction chunk
    x4 = nc.declare_dram_parameter("x4", [128, NT, KT, 512], BF16, isOutput=False)
    wq = nc.declare_dram_parameter("wq", [128, KT, 128], BF16, isOutput=False)
    wk = nc.declare_dram_parameter("wk", [128, KT, 128], BF16, isOutput=False)
    wv = nc.declare_dram_parameter("wv", [128, KT, 128], BF16, isOutput=False)
    wo = nc.declare_dram_parameter("wo", [HC, C], BF16, isOutput=False)
    if with_bias:
        bq = nc.declare_dram_parameter("bq", [HC, 1], F32, isOutput=False)
        bk = nc.declare_dram_parameter("bk", [HC, 1], F32, isOutput=False)
        bv = nc.declare_dram_parameter("bv", [HC, 1], F32, isOutput=False)
    masks = nc.declare_dram_parameter("masks", [128, HL, 128], BF16, isOutput=False)
    onesz = nc.declare_dram_parameter("onesz", [128, JB, D], BF16, isOutput=False)
    out = nc.declare_dram_parameter("out", [TOK, C], BF16, isOutput=True)

    Exp = mybir.ActivationFunctionType.Exp

    with contextlib.ExitStack() as _st:
        _st.enter_context(
            nc.allow_low_precision(reason="bf16 matmuls with fp32 accumulation")
        )
        tc = _st.enter_context(tile.TileContext(nc))
        with (
            tc.tile_pool(name="consts", bufs=1) as consts,
            tc.tile_pool(name="persist", bufs=1) as persist,
            tc.tile_pool(name="work", bufs=2) as work,
            tc.tile_pool(name="vap", bufs=4) as vap,
            tc.tile_pool(name="ps_qkv", bufs=2, space="PSUM") as ps_qkv,
            tc.tile_pool(name="ps_s", bufs=2, space="PSUM") as ps_s,
            tc.tile_pool(name="ps_o", bufs=2, space="PSUM") as ps_o,
        ):
            # ---- constants into SBUF ----
            wq_sb = consts.tile([128, KT, 128], BF16, name="wq_sb")
            wk_sb = consts.tile([128, KT, 128], BF16, name="wk_sb")
            wv_sb = consts.tile([128, KT, 128], BF16, name="wv_sb")
            for w_sb, w_dr in ((wq_sb, wq), (wk_sb, wk), (wv_sb, wv)):
                nc.sync.dma_start(w_sb, w_dr[:])
            wo_sb = consts.tile([128, C], BF16, name="wo_sb")
            nc.scalar.dma_start(wo_sb, wo[:])
            if with_bias:
                bq_sb = consts.tile([HC, 1], F32, name="bq_sb")
                bk_sb = consts.tile([HC, 1], F32, name="bk_sb")
                bv_sb = consts.tile([HC, 1], F32, name="bv_sb")
                for b_sb, b_dr in ((bq_sb, bq), (bk_sb, bk), (bv_sb, bv)):
                    nc.sync.dma_start(b_sb, b_dr[:])
                biases = (bq_sb, bk_sb, bv_sb)
            masks_sb = consts.tile([128, HL, 128], BF16, name="masks_sb")
            nc.scalar.dma_start(masks_sb, masks[:])
            onesz_sb = consts.tile([128, JB, D], BF16, name="onesz_sb")
            nc.scalar.dma_start(onesz_sb, onesz[:])

            # ---- persistent activations ----
            qT = persist.tile([HC, TOK], BF16, name="qT")
            kT = persist.tile([HC, TOK], BF16, name="kT")
            vT = persist.tile([HC, TOK], BF16, name="vT")
            attoT = persist.tile([HC, TOK], BF16, name="attoT")

            xchunks = []

            def x_load(nt):
                xchunk = work.tile(
                    [128, KT, 512], BF16, tag="xchunk", bufs=NT, name=f"xc{nt}"
                )
                nc.gpsimd.dma_start(xchunk, x4[:, nt])
                xchunks.append(xchunk)

            def a_group(nt):
                """QKV projections for one 512-token chunk."""
                c0 = nt * 512
                xchunk = xchunks[nt]
                for ti, (w_sb, dstT) in enumerate(
                    ((wq_sb, qT), (wk_sb, kT), (wv_sb, vT))
                ):
                    ps = ps_qkv.tile([128, 512], F32, tag="qkv")
                    for kt in range(KT):
                        nc.tensor.matmul(
                            ps,
                            lhsT=w_sb[:, kt, :],
                            rhs=xchunk[:, kt, :],
                            start=kt == 0,
                            stop=kt == KT - 1,
                        )
                    if with_bias:
                        nc.vector.tensor_scalar_add(
                            dstT[:, c0 : c0 + 512], ps, biases[ti]
                        )
                    else:
                        nc.vector.tensor_copy(dstT[:, c0 : c0 + 512], ps)

            def va_fill(va_tiles, b):
                """Fill cols 0..D-1 of va: col 0 ones (sums row), 1..D-1 zero."""
                for hl in range(HL):
                    nc.sync.dma_start(va_tiles[hl][:, :, 0:D], onesz_sb[:])

            def va_tr(va_tiles, b, g):
                """DMA-transpose one 512-token group of v into [tok, ch]."""
                t0 = b * T
                for hl in range(HL):
                    h0 = hl * D
                    nc.sync.dma_start(
                        va_tiles[hl][:, 4 * g : 4 * g + 4, D : 2 * D],
                        vT[h0 : h0 + D, t0 + 512 * g : t0 + 512 * (g + 1)],
                        transpose=True,
                    )

            pending_mul = []

            def flush_mul():
                while pending_mul:
                    pending_mul.pop(0)()

            def i_tile(b, i, va_tiles):
                """Attention for one 512-query tile, both local heads packed."""
                t0 = b * T
                q0 = t0 + i * 512
                njb = 4 * (i + 1)
                o_ps = [
                    ps_o.tile([128, 512], F32, tag="o", name=f"o{hl}")
                    for hl in range(HL)
                ]

                def scores(jb):
                    # diagonal block jb=4i+r: columns < 128r are fully masked
                    # and never computed or read; only the leading 128-wide
                    # sub-block needs the causal triangle
                    w0 = max(0, (jb - 4 * i) * 128)
                    s_pair = ps_s.tile([128, HL, 512], F32, tag="spair")
                    for hl in range(HL):
                        h0 = hl * D
                        nc.tensor.matmul(
                            s_pair[:, hl, w0:],
                            lhsT=kT[
                                h0 : h0 + D, t0 + jb * 128 : t0 + (jb + 1) * 128
                            ],
                            rhs=qT[h0 : h0 + D, q0 + w0 : q0 + 512],
                            start=True,
                            stop=True,
                            tile_position=(h0, 0),
                        )
                    e_pair = work.tile([128, HL, 512], BF16, tag="epair", bufs=6)
                    nc.scalar.activation(
                        e_pair[:, :, w0:], s_pair[:, :, w0:], Exp, scale=0.125
                    )
                    if jb >= 4 * i:
                        nc.gpsimd.tensor_mul(
                            e_pair[:, :, w0 : w0 + 128],
                            e_pair[:, :, w0 : w0 + 128],
                            masks_sb,
                        )
                    return e_pair, w0

                def attv(jb, e_pair, w0, start, stop):
                    # va col 0 is ones -> o_ps row 0 = exp row-sums; cols
                    # 1..D-1 are zero; v channels land on rows D..2D-1
                    for hl in range(HL):
                        nc.tensor.matmul(
                            o_ps[hl][:, w0:],
                            lhsT=va_tiles[hl][:, jb, :],
                            rhs=e_pair[:, hl, w0:],
                            start=start,
                            stop=stop,
                        )

                OFF = 1
                pend = []
                emitted = 0
                for jb in range(njb):
                    pend.append((jb, scores(jb)))
                    if len(pend) > OFF:
                        pj, (pe_, pw) = pend.pop(0)
                        attv(pj, pe_, pw, start=(emitted == 0),
                             stop=(emitted == njb - 1))
                        emitted += 1
                for pj, (pe_, pw) in pend:
                    attv(pj, pe_, pw, start=(emitted == 0),
                         stop=(emitted == njb - 1))
                    emitted += 1

                # normalize: 1/rowsum from the ones column, replicated over
                # partitions by an SBUF->SBUF broadcast DMA, scale into attoT
                # normalize: copy channels to attoT unnormalized (frees the
                # PSUM bank fast), recip of the sums row on DVE, broadcast it
                # across partitions with an SWDGE DMA, then one in-place Pool
                # mul covering both heads — no PE or ACT work at all
                recips = [
                    work.tile([1, 1, 512], F32, tag=f"recips{hl}", name=f"recips{hl}")
                    for hl in range(HL)
                ]
                rb_sb = work.tile([128, 512], F32, tag="rb")
                for hl in range(HL):
                    h0 = hl * D
                    nc.vector.tensor_copy(
                        attoT[h0 : h0 + D, q0 : q0 + 512], o_ps[hl][D : 2 * D, :]
                    )
                    nc.vector.reciprocal_approx_fast(
                        recips[hl][:, 0, :], o_ps[hl][0:1, :]
                    )
                    nc.sync.dma_start(
                        rb_sb[h0 : h0 + D, :], recips[hl].to_broadcast([1, D, 512])
                    )
                flush_mul()
                pending_mul.append(
                    lambda q0=q0, rb_sb=rb_sb: nc.gpsimd.tensor_mul(
                        attoT[:, q0 : q0 + 512], attoT[:, q0 : q0 + 512], rb_sb
                    )
                )

            def c_group(tt, copy_eng):
                """Output projection for one 128-token block + bf16 store."""
                o_sb = work.tile([128, C], BF16, tag="osb", bufs=3)
                for no2 in range(2):
                    p_ps = ps_qkv.tile([128, 512], F32, tag="qkv")
                    nc.tensor.matmul(
                        p_ps,
                        lhsT=attoT[:, tt * 128 : (tt + 1) * 128],
                        rhs=wo_sb[:, no2 * 512 : (no2 + 1) * 512],
                        start=True,
                        stop=True,
                    )
                    if copy_eng == "scalar":
                        nc.scalar.copy(o_sb[:, no2 * 512 : (no2 + 1) * 512], p_ps)
                    else:
                        nc.vector.tensor_copy(
                            o_sb[:, no2 * 512 : (no2 + 1) * 512], p_ps
                        )
                nc.sync.dma_start(out[tt * 128 : (tt + 1) * 128, :], o_sb)

            # ---- pipelined emission ----
            # all x chunks stream in on the scalar HWDGE ring from the start
            for nt in range(NT):
                x_load(nt)
            va0 = [
                vap.tile([128, JB, 2 * D], BF16, tag="va", name=f"va0_{hl}")
                for hl in range(HL)
            ]
            va_fill(va0, 0)
            _s1 = nc.enter_named_scope("W1", True)
            a_group(0)
            va_tr(va0, 0, 0)
            for i in range(QT):
                if i + 1 < QT:
                    a_group(i + 1)
                    va_tr(va0, 0, i + 1)
                i_tile(0, i, va0)
            nc.leave_named_scope("W1", _s1[0], True)

            _s2 = nc.enter_named_scope("W2", True)
            va1 = [
                vap.tile([128, JB, 2 * D], BF16, tag="va", name=f"va1_{hl}")
                for hl in range(HL)
            ]
            va_fill(va1, 1)
            a_group(QT)
            va_tr(va1, 1, 0)
            for i in range(QT):
                if i + 1 < QT:
                    a_group(QT + i + 1)
                    va_tr(va1, 1, i + 1)
                for tt in range(4 * i, 4 * i + 4):
                    c_group(tt, "vector")
                i_tile(1, i, va1)
                if i >= 1:
                    # batch-1 output projection lags its i-tile by one slot
                    for tt in range(JB + 4 * (i - 1), JB + 4 * i):
                        c_group(tt, "vector")
            nc.leave_named_scope("W2", _s2[0], True)

            _s3 = nc.enter_named_scope("W3", True)
            flush_mul()
            for tt in range(2 * JB - 4, 2 * JB):
                c_group(tt, "vector")
            nc.leave_named_scope("W3", _s3[0], True)

    _split_waits(nc)
    # populate .instr bytes for custom-DVE InstISA (reciprocal_approx_fast);
    # raw Bass skips this pass and the NEFF compiler then sees "ISA wrong
    # length"
    from concourse.library_overlay import lower_extended_insts

    lower_extended_insts(nc)
    return nc


def make_in_maps(x, Wq, bq, Wk, bk, Wv, bv, Wo, bo, with_bias):
    xT = np.ascontiguousarray(x.reshape(TOK, C).T).astype(NPBF16)
    # x4[p, nt, a, m] = x.T[a*128 + p, nt*512 + m]
    x4 = np.ascontiguousarray(
        xT.reshape(KT, 128, NT, 512).transpose(1, 2, 0, 3)
    )
    # single causal triangle [128, HL, 128]: mask[p, :, c] = 1 if c >= p
    a = np.arange(128)[:, None]
    c = np.arange(128)[None, :]
    masks = np.ascontiguousarray(
        np.repeat((c >= a).astype(NPBF16)[:, None, :], HL, axis=1)
    )
    onesz = np.zeros((128, JB, D), NPBF16)
    onesz[:, :, 0] = 1.0
    in_maps = []
    for core in range(NCORES):
        sl = slice(core * HC, (core + 1) * HC)
        def warr(W):
            # [128, KT, 128]: w3[p, a, m] = W.T[a*128 + p, m]
            return np.ascontiguousarray(
                W[sl, :].T.astype(NPBF16).reshape(KT, 128, HC).transpose(1, 0, 2)
            )

        m = {
            "x4": x4,
            "wq": warr(Wq),
            "wk": warr(Wk),
            "wv": warr(Wv),
            "wo": np.ascontiguousarray(Wo[:, sl].T).astype(NPBF16),
            "masks": masks,
            "onesz": onesz,
        }
        if with_bias:
            m["bq"] = np.ascontiguousarray(bq[sl]).reshape(HC, 1).astype(np.float32)
            m["bk"] = np.ascontiguousarray(bk[sl]).reshape(HC, 1).astype(np.float32)
            m["bv"] = np.ascontiguousarray(bv[sl]).reshape(HC, 1).astype(np.float32)
        in_maps.append(m)
    return in_maps


_NC_CACHE = {}


def kernel(x, Wq, bq, Wk, bk, Wv, bv, Wo, bo):
    x = np.asarray(x, np.float32)
    bq = np.asarray(bq, np.float32)
    bk = np.asarray(bk, np.float32)
    bv = np.asarray(bv, np.float32)
    with_bias = bool(np.any(bq) or np.any(bk) or np.any(bv))
    in_maps = make_in_maps(
        x,
        np.asarray(Wq, np.float32),
        bq,
        np.asarray(Wk, np.float32),
        bk,
        np.asarray(Wv, np.float32),
        bv,
        np.asarray(Wo, np.float32),
        np.asarray(bo, np.float32),
        with_bias,
    )
    if with_bias not in _NC_CACHE:
        _NC_CACHE[with_bias] = build(with_bias)
    trace = bool(int(os.environ.get("KERNEL_TRACE", "0")))
    res = run_bass_kernel_spmd(
        _NC_CACHE[with_bias], in_maps, core_ids=list(range(NCORES)), trace=trace
    )
    if trace:
        kernel.last_results = res
    total = np.zeros((TOK, C), np.float32)
    for core in range(NCORES):
        total += res.results[core]["out"].astype(np.float32)
    total += np.asarray(bo, np.float32)[None, :]
    return total.reshape(B, T, C)


# revision 28
# speedup vs baseline: 1.0417x; 1.0417x over previous
"""Multi-head causal attention (B=2, T=2048, C=1024, H=16) on 8 trn2 cores.

Sharding: tensor-parallel over heads. Each core computes 2 heads' QKV
projections + attention + a partial output projection; the host sums the
8 partial projections and adds the output bias.

v2: pipelined emission (QKV-projection groups interleaved with attention
i-tiles so the PE never drains), per-i-tile softmax normalization via
reciprocal_approx_fast + a K=2 broadcast matmul (replaces the serial
[1,2048] DVE reciprocal that idled the PE past the HAM window), 2-head
score matmuls packed into one PE slot via row tiling, exp merged over
both heads' PSUM banks, mask-muls on the idle GpSimd engine, bf16
partial outputs.
"""

import contextlib
import os

import ml_dtypes
import numpy as np

import bass_rust
import concourse.bass as bass
import concourse.mybir as mybir
import concourse.tile as tile
from concourse.bass_utils import run_bass_kernel_spmd

F32 = mybir.dt.float32
F32R = mybir.dt.float32r
BF16 = mybir.dt.bfloat16
NPBF16 = ml_dtypes.bfloat16

B, T, C, H = 2, 2048, 1024, 16
D = C // H          # 64
NCORES = 8
HL = H // NCORES    # heads per core = 2
TOK = B * T         # 4096
HC = HL * D         # local head channels = 128

NT = TOK // 512     # 8 token column tiles (512) over both batches
KT = C // 128       # 8 contraction tiles for projections
QT = T // 512       # 4 q tiles per batch
JB = T // 128       # 16 j (key) blocks per batch

_MAXW = 1


def _patched_drain_and_barrier(self, tick_clock, wait_clock):
    """Stock tile tail drain carries one sem-wait per outstanding proc on a
    single TPB_CTRL drain; this walrus build allows only one sync-wait per
    ctrl instruction. Split the waits across no-op carriers."""
    nc = self.nc
    carrier = nc.sync.nop()
    wait_clock.add_sem_waits(
        carrier.ins, bass_rust.ScopedClock({None: tick_clock.global_clock})
    )
    si = carrier.ins.sync_info
    waits = list(si.on_wait) if si and si.on_wait else []
    if len(waits) > _MAXW:
        carrier.ins.sync_info = mybir.SyncInfo(
            on_wait=waits[:_MAXW], on_update=list(si.on_update or [])
        )
        for i in range(_MAXW, len(waits), _MAXW):
            nop = nc.sync.nop()
            nop.ins.sync_info = mybir.SyncInfo(
                on_wait=waits[i : i + _MAXW], on_update=[]
            )
    nc.sync.drain()

    nc.all_engine_barrier()
    popped = nc._tile_sem_poison_stack.pop()
    assert popped is self._sem_poison
    assert self.sems is not None
    nc.clear_and_free_semaphores(list(self.sems.allocated().values()))
    nc.all_engine_barrier()


tile.TileContext._drain_and_barrier = _patched_drain_and_barrier


def _split_waits(nc, maxw=_MAXW):
    """This walrus build accepts at most one sync-wait per instruction.
    Hoist excess waits onto no-op carriers inserted just before the
    instruction on the same engine."""
    for f in nc.m.functions:
        for bb in f.blocks:
            insts = bb.instructions
            if not any(
                i.sync_info and i.sync_info.on_wait and len(i.sync_info.on_wait) > maxw
                for i in insts
            ):
                continue
            new = []
            for inst in insts:
                si = inst.sync_info
                waits = list(si.on_wait) if si and si.on_wait else []
                if len(waits) > maxw:
                    keep = waits[-maxw:]
                    extra = waits[:-maxw]
                    for j in range(0, len(extra), maxw):
                        nop = mybir.InstNoOp(name=nc.get_next_instruction_name())
                        nop.engine = inst.engine
                        nop.sync_info = mybir.SyncInfo(
                            on_wait=extra[j : j + maxw], on_update=[]
                        )
                        nc.register_instruction(nop)
                        new.append(nop)
                    inst.sync_info = mybir.SyncInfo(
                        on_wait=keep, on_update=list(si.on_update or [])
                    )
                new.append(inst)
            bb.instructions = new


def build(with_bias):
    nc = bass.Bass()
    # x3[p, a, m] = x.T[a*128 + p, m] — pre-rearranged on host so one DMA
    # fetches a [128, 8, 512] contraction chunk
    x4 = nc.declare_dram_parameter("x4", [128, NT, KT, 512], BF16, isOutput=False)
    wq = nc.declare_dram_parameter("wq", [128, KT, 128], BF16, isOutput=False)
    wk = nc.declare_dram_parameter("wk", [128, KT, 128], BF16, isOutput=False)
    wv = nc.declare_dram_parameter("wv", [128, KT, 128], BF16, isOutput=False)
    wo = nc.declare_dram_parameter("wo", [HC, C], BF16, isOutput=False)
    if with_bias:
        bq = nc.declare_dram_parameter("bq", [HC, 1], F32, isOutput=False)
        bk = nc.declare_dram_parameter("bk", [HC, 1], F32, isOutput=False)
        bv = nc.declare_dram_parameter("bv", [HC, 1], F32, isOutput=False)
    masks = nc.declare_dram_parameter("masks", [128, HL, 128], BF16, isOutput=False)
    onesz = nc.declare_dram_parameter("onesz", [128, JB, D], BF16, isOutput=False)
    out = nc.declare_dram_parameter("out", [TOK, C], BF16, isOutput=True)

    Exp = mybir.ActivationFunctionType.Exp

    with contextlib.ExitStack() as _st:
        _st.enter_context(
            nc.allow_low_precision(reason="bf16 matmuls with fp32 accumulation")
        )
        tc = _st.enter_context(tile.TileContext(nc))
        with (
            tc.tile_pool(name="consts", bufs=1) as consts,
            tc.tile_pool(name="persist", bufs=1) as persist,
            tc.tile_pool(name="work", bufs=2) as work,
            tc.tile_pool(name="vap", bufs=4) as vap,
            tc.tile_pool(name="ps_qkv", bufs=2, space="PSUM") as ps_qkv,
            tc.tile_pool(name="ps_s", bufs=2, space="PSUM") as ps_s,
            tc.tile_pool(name="ps_o", bufs=2, space="PSUM") as ps_o,
        ):
            # ---- constants into SBUF ----
            wq_sb = consts.tile([128, KT, 128], BF16, name="wq_sb")
            wk_sb = consts.tile([128, KT, 128], BF16, name="wk_sb")
            wv_sb = consts.tile([128, KT, 128], BF16, name="wv_sb")
            for w_sb, w_dr in ((wq_sb, wq), (wk_sb, wk), (wv_sb, wv)):
                nc.sync.dma_start(w_sb, w_dr[:])
            wo_sb = consts.tile([128, C], BF16, name="wo_sb")
            nc.scalar.dma_start(wo_sb, wo[:])
            if with_bias:
                bq_sb = consts.tile([HC, 1], F32, name="bq_sb")
                bk_sb = consts.tile([HC, 1], F32, name="bk_sb")
                bv_sb = consts.tile([HC, 1], F32, name="bv_sb")
                for b_sb, b_dr in ((bq_sb, bq), (bk_sb, bk), (bv_sb, bv)):
                    nc.sync.dma_start(b_sb, b_dr[:])
                biases = (bq_sb, bk_sb, bv_sb)
            masks_sb = consts.tile([128, HL, 128], BF16, name="masks_sb")
            nc.scalar.dma_start(masks_sb, masks[:])
            onesz_sb = consts.tile([128, JB, D], BF16, name="onesz_sb")
            nc.scalar.dma_start(onesz_sb, onesz[:])

            # ---- persistent activations ----
            qT = persist.tile([HC, TOK], BF16, name="qT")
            kT = persist.tile([HC, TOK], BF16, name="kT")
            vT = persist.tile([HC, TOK], BF16, name="vT")
            attoT = persist.tile([HC, TOK], BF16, name="attoT")

            xchunks = []

            def x_load(nt):
                xchunk = work.tile(
                    [128, KT, 512], BF16, tag="xchunk", bufs=NT, name=f"xc{nt}"
                )
                nc.gpsimd.dma_start(xchunk, x4[:, nt])
                xchunks.append(xchunk)

            def a_group(nt):
                """QKV projections for one 512-token chunk."""
                c0 = nt * 512
                xchunk = xchunks[nt]
                for ti, (w_sb, dstT) in enumerate(
                    ((wq_sb, qT), (wk_sb, kT), (wv_sb, vT))
                ):
                    ps = ps_qkv.tile([128, 512], F32, tag="qkv")
                    for kt in range(KT):
                        nc.tensor.matmul(
                            ps,
                            lhsT=w_sb[:, kt, :],
                            rhs=xchunk[:, kt, :],
                            start=kt == 0,
                            stop=kt == KT - 1,
                        )
                    if with_bias:
                        nc.vector.tensor_scalar_add(
                            dstT[:, c0 : c0 + 512], ps, biases[ti]
                        )
                    else:
                        nc.vector.tensor_copy(dstT[:, c0 : c0 + 512], ps)

            def va_fill(va_tiles, b):
                """Fill cols 0..D-1 of va: col 0 ones (sums row), 1..D-1 zero."""
                for hl in range(HL):
                    nc.sync.dma_start(va_tiles[hl][:, :, 0:D], onesz_sb[:])

            def va_tr(va_tiles, b, g):
                """DMA-transpose one 512-token group of v into [tok, ch]."""
                t0 = b * T
                for hl in range(HL):
                    h0 = hl * D
                    nc.sync.dma_start(
                        va_tiles[hl][:, 4 * g : 4 * g + 4, D : 2 * D],
                        vT[h0 : h0 + D, t0 + 512 * g : t0 + 512 * (g + 1)],
                        transpose=True,
                    )

            pending_mul = []

            def flush_mul():
                while pending_mul:
                    pending_mul.pop(0)()

            def i_tile(b, i, va_tiles):
                """Attention for one 512-query tile, both local heads packed."""
                t0 = b * T
                q0 = t0 + i * 512
                njb = 4 * (i + 1)
                o_ps = [
                    ps_o.tile([128, 512], F32, tag="o", name=f"o{hl}")
                    for hl in range(HL)
                ]

                def scores(jb):
                    # diagonal block jb=4i+r: columns < 128r are fully masked
                    # and never computed or read; only the leading 128-wide
                    # sub-block needs the causal triangle
                    w0 = max(0, (jb - 4 * i) * 128)
                    s_pair = ps_s.tile([128, HL, 512], F32, tag="spair")
                    for hl in range(HL):
                        h0 = hl * D
                        nc.tensor.matmul(
                            s_pair[:, hl, w0:],
                            lhsT=kT[
                                h0 : h0 + D, t0 + jb * 128 : t0 + (jb + 1) * 128
                            ],
                            rhs=qT[h0 : h0 + D, q0 + w0 : q0 + 512],
                            start=True,
                            stop=True,
                            tile_position=(h0, 0),
                        )
                    e_pair = work.tile([128, HL, 512], BF16, tag="epair", bufs=6)
                    nc.scalar.activation(
                        e_pair[:, :, w0:], s_pair[:, :, w0:], Exp, scale=0.125
                    )
                    if jb >= 4 * i:
                        nc.gpsimd.tensor_mul(
                            e_pair[:, :, w0 : w0 + 128],
                            e_pair[:, :, w0 : w0 + 128],
                            masks_sb,
                        )
                    return e_pair, w0

                def attv(jb, e_pair, w0, start, stop):
                    # va col 0 is ones -> o_ps row 0 = exp row-sums; cols
                    # 1..D-1 are zero; v channels land on rows D..2D-1
                    for hl in range(HL):
                        nc.tensor.matmul(
                            o_ps[hl][:, w0:],
                            lhsT=va_tiles[hl][:, jb, :],
                            rhs=e_pair[:, hl, w0:],
                            start=start,
                            stop=stop,
                        )

                OFF = 2
                pend = []
                emitted = 0
                for jb in range(njb):
                    pend.append((jb, scores(jb)))
                    if len(pend) > OFF:
                        pj, (pe_, pw) = pend.pop(0)
                        attv(pj, pe_, pw, start=(emitted == 0),
                             stop=(emitted == njb - 1))
                        emitted += 1
                for pj, (pe_, pw) in pend:
                    attv(pj, pe_, pw, start=(emitted == 0),
                         stop=(emitted == njb - 1))
                    emitted += 1

                # normalize: 1/rowsum from the ones column, replicated over
                # partitions by an SBUF->SBUF broadcast DMA, scale into attoT
                # normalize: copy channels to attoT unnormalized (frees the
                # PSUM bank fast), recip of the sums row on DVE, broadcast it
                # across partitions with an SWDGE DMA, then one in-place Pool
                # mul covering both heads — no PE or ACT work at all
                recips = [
                    work.tile([1, 1, 512], F32, tag=f"recips{hl}", name=f"recips{hl}")
                    for hl in range(HL)
                ]
                rb_sb = work.tile([128, 512], F32, tag="rb")
                for hl in range(HL):
                    h0 = hl * D
                    nc.vector.tensor_copy(
                        attoT[h0 : h0 + D, q0 : q0 + 512], o_ps[hl][D : 2 * D, :]
                    )
                    nc.vector.reciprocal_approx_fast(
                        recips[hl][:, 0, :], o_ps[hl][0:1, :]
                    )
                    nc.sync.dma_start(
                        rb_sb[h0 : h0 + D, :], recips[hl].to_broadcast([1, D, 512])
                    )
                flush_mul()
                pending_mul.append(
                    lambda q0=q0, rb_sb=rb_sb: nc.gpsimd.tensor_mul(
                        attoT[:, q0 : q0 + 512], attoT[:, q0 : q0 + 512], rb_sb
                    )
                )

            def c_group(tt, copy_eng):
                """Output projection for one 128-token block + bf16 store."""
                o_sb = work.tile([128, C], BF16, tag="osb", bufs=3)
                for no2 in range(2):
                    p_ps = ps_qkv.tile([128, 512], F32, tag="qkv")
                    nc.tensor.matmul(
                        p_ps,
                        lhsT=attoT[:, tt * 128 : (tt + 1) * 128],
                        rhs=wo_sb[:, no2 * 512 : (no2 + 1) * 512],
                        start=True,
                        stop=True,
                    )
                    if copy_eng == "scalar":
                        nc.scalar.copy(o_sb[:, no2 * 512 : (no2 + 1) * 512], p_ps)
                    else:
                        nc.vector.tensor_copy(
                            o_sb[:, no2 * 512 : (no2 + 1) * 512], p_ps
                        )
                nc.sync.dma_start(out[tt * 128 : (tt + 1) * 128, :], o_sb)

            # ---- pipelined emission ----
            # all x chunks stream in on the scalar HWDGE ring from the start
            for nt in range(NT):
                x_load(nt)
            va0 = [
                vap.tile([128, JB, 2 * D], BF16, tag="va", name=f"va0_{hl}")
                for hl in range(HL)
            ]
            va_fill(va0, 0)
            va1 = [
                vap.tile([128, JB, 2 * D], BF16, tag="va", name=f"va1_{hl}")
                for hl in range(HL)
            ]
            va_fill(va1, 1)
            _s1 = nc.enter_named_scope("W1", True)
            a_group(0)
            va_tr(va0, 0, 0)
            for i in range(QT):
                if i + 1 < QT:
                    a_group(i + 1)
                    va_tr(va0, 0, i + 1)
                a_group(QT + i)
                va_tr(va1, 1, i)
                i_tile(0, i, va0)
            nc.leave_named_scope("W1", _s1[0], True)

            _s2 = nc.enter_named_scope("W2", True)
            for i in range(QT):
                for tt in range(4 * i, 4 * i + 4):
                    c_group(tt, "vector")
                i_tile(1, i, va1)
                if i >= 1:
                    # batch-1 output projection lags its i-tile by one slot
                    for tt in range(JB + 4 * (i - 1), JB + 4 * i):
                        c_group(tt, "vector")
            nc.leave_named_scope("W2", _s2[0], True)

            _s3 = nc.enter_named_scope("W3", True)
            flush_mul()
            for tt in range(2 * JB - 4, 2 * JB):
                c_group(tt, "vector")
            nc.leave_named_scope("W3", _s3[0], True)

    _split_waits(nc)
    # populate .instr bytes for custom-DVE InstISA (reciprocal_approx_fast);
    # raw Bass skips this pass and the NEFF compiler then sees "ISA wrong
    # length"
    from concourse.library_overlay import lower_extended_insts

    lower_extended_insts(nc)
    return nc


def make_in_maps(x, Wq, bq, Wk, bk, Wv, bv, Wo, bo, with_bias):
    xT = np.ascontiguousarray(x.reshape(TOK, C).T).astype(NPBF16)
    # x4[p, nt, a, m] = x.T[a*128 + p, nt*512 + m]
    x4 = np.ascontiguousarray(
        xT.reshape(KT, 128, NT, 512).transpose(1, 2, 0, 3)
    )
    # single causal triangle [128, HL, 128]: mask[p, :, c] = 1 if c >= p
    a = np.arange(128)[:, None]
    c = np.arange(128)[None, :]
    masks = np.ascontiguousarray(
        np.repeat((c >= a).astype(NPBF16)[:, None, :], HL, axis=1)
    )
    onesz = np.zeros((128, JB, D), NPBF16)
    onesz[:, :, 0] = 1.0
    in_maps = []
    for core in range(NCORES):
        sl = slice(core * HC, (core + 1) * HC)
        def warr(W):
            # [128, KT, 128]: w3[p, a, m] = W.T[a*128 + p, m]
            return np.ascontiguousarray(
                W[sl, :].T.astype(NPBF16).reshape(KT, 128, HC).transpose(1, 0, 2)
            )

        m = {
            "x4": x4,
            "wq": warr(Wq),
            "wk": warr(Wk),
            "wv": warr(Wv),
            "wo": np.ascontiguousarray(Wo[:, sl].T).astype(NPBF16),
            "masks": masks,
            "onesz": onesz,
        }
        if with_bias:
            m["bq"] = np.ascontiguousarray(bq[sl]).reshape(HC, 1).astype(np.float32)
            m["bk"] = np.ascontiguousarray(bk[sl]).reshape(HC, 1).astype(np.float32)
            m["bv"] = np.ascontiguousarray(bv[sl]).reshape(HC, 1).astype(np.float32)
        in_maps.append(m)
    return in_maps


_NC_CACHE = {}


def kernel(x, Wq, bq, Wk, bk, Wv, bv, Wo, bo):
    x = np.asarray(x, np.float32)
    bq = np.asarray(bq, np.float32)
    bk = np.asarray(bk, np.float32)
    bv = np.asarray(bv, np.float32)
    with_bias = bool(np.any(bq) or np.any(bk) or np.any(bv))
    in_maps = make_in_maps(
        x,
        np.asarray(Wq, np.float32),
        bq,
        np.asarray(Wk, np.float32),
        bk,
        np.asarray(Wv, np.float32),
        bv,
        np.asarray(Wo, np.float32),
        np.asarray(bo, np.float32),
        with_bias,
    )
    if with_bias not in _NC_CACHE:
        _NC_CACHE[with_bias] = build(with_bias)
    trace = bool(int(os.environ.get("KERNEL_TRACE", "0")))
    res = run_bass_kernel_spmd(
        _NC_CACHE[with_bias], in_maps, core_ids=list(range(NCORES)), trace=trace
    )
    if trace:
        kernel.last_results = res
    total = np.zeros((TOK, C), np.float32)
    for core in range(NCORES):
        total += res.results[core]["out"].astype(np.float32)
    total += np.asarray(bo, np.float32)[None, :]
    return total.reshape(B, T, C)


# revision 29
# speedup vs baseline: 1.0476x; 1.0057x over previous
"""Multi-head causal attention (B=2, T=2048, C=1024, H=16) on 8 trn2 cores.

Sharding: tensor-parallel over heads. Each core computes 2 heads' QKV
projections + attention + a partial output projection; the host sums the
8 partial projections and adds the output bias.

v2: pipelined emission (QKV-projection groups interleaved with attention
i-tiles so the PE never drains), per-i-tile softmax normalization via
reciprocal_approx_fast + a K=2 broadcast matmul (replaces the serial
[1,2048] DVE reciprocal that idled the PE past the HAM window), 2-head
score matmuls packed into one PE slot via row tiling, exp merged over
both heads' PSUM banks, mask-muls on the idle GpSimd engine, bf16
partial outputs.
"""

import contextlib
import os

import ml_dtypes
import numpy as np

import bass_rust
import concourse.bass as bass
import concourse.mybir as mybir
import concourse.tile as tile
from concourse.bass_utils import run_bass_kernel_spmd

F32 = mybir.dt.float32
F32R = mybir.dt.float32r
BF16 = mybir.dt.bfloat16
NPBF16 = ml_dtypes.bfloat16

B, T, C, H = 2, 2048, 1024, 16
D = C // H          # 64
NCORES = 8
HL = H // NCORES    # heads per core = 2
TOK = B * T         # 4096
HC = HL * D         # local head channels = 128

NT = TOK // 512     # 8 token column tiles (512) over both batches
KT = C // 128       # 8 contraction tiles for projections
QT = T // 512       # 4 q tiles per batch
JB = T // 128       # 16 j (key) blocks per batch

_MAXW = 1


def _patched_drain_and_barrier(self, tick_clock, wait_clock):
    """Stock tile tail drain carries one sem-wait per outstanding proc on a
    single TPB_CTRL drain; this walrus build allows only one sync-wait per
    ctrl instruction. Split the waits across no-op carriers."""
    nc = self.nc
    carrier = nc.sync.nop()
    wait_clock.add_sem_waits(
        carrier.ins, bass_rust.ScopedClock({None: tick_clock.global_clock})
    )
    si = carrier.ins.sync_info
    waits = list(si.on_wait) if si and si.on_wait else []
    if len(waits) > _MAXW:
        carrier.ins.sync_info = mybir.SyncInfo(
            on_wait=waits[:_MAXW], on_update=list(si.on_update or [])
        )
        for i in range(_MAXW, len(waits), _MAXW):
            nop = nc.sync.nop()
            nop.ins.sync_info = mybir.SyncInfo(
                on_wait=waits[i : i + _MAXW], on_update=[]
            )
    nc.sync.drain()

    nc.all_engine_barrier()
    popped = nc._tile_sem_poison_stack.pop()
    assert popped is self._sem_poison
    assert self.sems is not None
    nc.clear_and_free_semaphores(list(self.sems.allocated().values()))
    nc.all_engine_barrier()


tile.TileContext._drain_and_barrier = _patched_drain_and_barrier


def _split_waits(nc, maxw=_MAXW):
    """This walrus build accepts at most one sync-wait per instruction.
    Hoist excess waits onto no-op carriers inserted just before the
    instruction on the same engine."""
    for f in nc.m.functions:
        for bb in f.blocks:
            insts = bb.instructions
            if not any(
                i.sync_info and i.sync_info.on_wait and len(i.sync_info.on_wait) > maxw
                for i in insts
            ):
                continue
            new = []
            for inst in insts:
                si = inst.sync_info
                waits = list(si.on_wait) if si and si.on_wait else []
                if len(waits) > maxw:
                    keep = waits[-maxw:]
                    extra = waits[:-maxw]
                    for j in range(0, len(extra), maxw):
                        nop = mybir.InstNoOp(name=nc.get_next_instruction_name())
                        nop.engine = inst.engine
                        nop.sync_info = mybir.SyncInfo(
                            on_wait=extra[j : j + maxw], on_update=[]
                        )
                        nc.register_instruction(nop)
                        new.append(nop)
                    inst.sync_info = mybir.SyncInfo(
                        on_wait=keep, on_update=list(si.on_update or [])
                    )
                new.append(inst)
            bb.instructions = new


def build(with_bias):
    nc = bass.Bass()
    # x3[p, a, m] = x.T[a*128 + p, m] — pre-rearranged on host so one DMA
    # fetches a [128, 8, 512] contraction chunk
    x4 = nc.declare_dram_parameter("x4", [128, NT, KT, 512], BF16, isOutput=False)
    wq = nc.declare_dram_parameter("wq", [128, KT, 128], BF16, isOutput=False)
    wk = nc.declare_dram_parameter("wk", [128, KT, 128], BF16, isOutput=False)
    wv = nc.declare_dram_parameter("wv", [128, KT, 128], BF16, isOutput=False)
    wo = nc.declare_dram_parameter("wo", [HC, C], BF16, isOutput=False)
    if with_bias:
        bq = nc.declare_dram_parameter("bq", [HC, 1], F32, isOutput=False)
        bk = nc.declare_dram_parameter("bk", [HC, 1], F32, isOutput=False)
        bv = nc.declare_dram_parameter("bv", [HC, 1], F32, isOutput=False)
    masks = nc.declare_dram_parameter("masks", [128, HL, 128], BF16, isOutput=False)
    onesz = nc.declare_dram_parameter("onesz", [128, JB, D], BF16, isOutput=False)
    out = nc.declare_dram_parameter("out", [TOK, C], BF16, isOutput=True)

    Exp = mybir.ActivationFunctionType.Exp

    with contextlib.ExitStack() as _st:
        _st.enter_context(
            nc.allow_low_precision(reason="bf16 matmuls with fp32 accumulation")
        )
        tc = _st.enter_context(tile.TileContext(nc))
        with (
            tc.tile_pool(name="consts", bufs=1) as consts,
            tc.tile_pool(name="persist", bufs=1) as persist,
            tc.tile_pool(name="work", bufs=2) as work,
            tc.tile_pool(name="vap", bufs=4) as vap,
            tc.tile_pool(name="ps_qkv", bufs=2, space="PSUM") as ps_qkv,
            tc.tile_pool(name="ps_s", bufs=2, space="PSUM") as ps_s,
            tc.tile_pool(name="ps_o", bufs=2, space="PSUM") as ps_o,
        ):
            # ---- constants into SBUF ----
            wq_sb = consts.tile([128, KT, 128], BF16, name="wq_sb")
            wk_sb = consts.tile([128, KT, 128], BF16, name="wk_sb")
            wv_sb = consts.tile([128, KT, 128], BF16, name="wv_sb")
            for w_sb, w_dr in ((wq_sb, wq), (wk_sb, wk), (wv_sb, wv)):
                nc.sync.dma_start(w_sb, w_dr[:])
            wo_sb = consts.tile([128, C], BF16, name="wo_sb")
            nc.sync.dma_start(wo_sb, wo[:])
            if with_bias:
                bq_sb = consts.tile([HC, 1], F32, name="bq_sb")
                bk_sb = consts.tile([HC, 1], F32, name="bk_sb")
                bv_sb = consts.tile([HC, 1], F32, name="bv_sb")
                for b_sb, b_dr in ((bq_sb, bq), (bk_sb, bk), (bv_sb, bv)):
                    nc.sync.dma_start(b_sb, b_dr[:])
                biases = (bq_sb, bk_sb, bv_sb)
            masks_sb = consts.tile([128, HL, 128], BF16, name="masks_sb")
            nc.scalar.dma_start(masks_sb, masks[:])
            onesz_sb = consts.tile([128, JB, D], BF16, name="onesz_sb")
            nc.scalar.dma_start(onesz_sb, onesz[:])

            # ---- persistent activations ----
            qT = persist.tile([HC, TOK], BF16, name="qT")
            kT = persist.tile([HC, TOK], BF16, name="kT")
            vT = persist.tile([HC, TOK], BF16, name="vT")
            attoT = persist.tile([HC, TOK], BF16, name="attoT")

            xchunks = []

            def x_load(nt):
                xchunk = work.tile(
                    [128, KT, 512], BF16, tag="xchunk", bufs=NT, name=f"xc{nt}"
                )
                # chunk 0 on the sync ring (first, ahead of weights); the rest
                # stream on the scalar HWDGE ring before any exps are queued
                if nt == 0:
                    nc.sync.dma_start(xchunk, x4[:, nt])
                else:
                    nc.scalar.dma_start(xchunk, x4[:, nt])
                xchunks.append(xchunk)

            def a_group(nt):
                """QKV projections for one 512-token chunk."""
                c0 = nt * 512
                xchunk = xchunks[nt]
                for ti, (w_sb, dstT) in enumerate(
                    ((wq_sb, qT), (wk_sb, kT), (wv_sb, vT))
                ):
                    ps = ps_qkv.tile([128, 512], F32, tag="qkv")
                    for kt in range(KT):
                        nc.tensor.matmul(
                            ps,
                            lhsT=w_sb[:, kt, :],
                            rhs=xchunk[:, kt, :],
                            start=kt == 0,
                            stop=kt == KT - 1,
                        )
                    if with_bias:
                        nc.vector.tensor_scalar_add(
                            dstT[:, c0 : c0 + 512], ps, biases[ti]
                        )
                    else:
                        nc.vector.tensor_copy(dstT[:, c0 : c0 + 512], ps)

            def va_fill(va_tiles, b):
                """Fill cols 0..D-1 of va: col 0 ones (sums row), 1..D-1 zero."""
                for hl in range(HL):
                    nc.sync.dma_start(va_tiles[hl][:, :, 0:D], onesz_sb[:])

            def va_tr(va_tiles, b, g):
                """DMA-transpose one 512-token group of v into [tok, ch]."""
                t0 = b * T
                for hl in range(HL):
                    h0 = hl * D
                    nc.sync.dma_start(
                        va_tiles[hl][:, 4 * g : 4 * g + 4, D : 2 * D],
                        vT[h0 : h0 + D, t0 + 512 * g : t0 + 512 * (g + 1)],
                        transpose=True,
                    )

            pending_mul = []

            def flush_mul():
                while pending_mul:
                    pending_mul.pop(0)()

            def i_tile(b, i, va_tiles):
                """Attention for one 512-query tile, both local heads packed."""
                t0 = b * T
                q0 = t0 + i * 512
                njb = 4 * (i + 1)
                o_ps = [
                    ps_o.tile([128, 512], F32, tag="o", name=f"o{hl}")
                    for hl in range(HL)
                ]

                def scores(jb):
                    # diagonal block jb=4i+r: columns < 128r are fully masked
                    # and never computed or read; only the leading 128-wide
                    # sub-block needs the causal triangle
                    w0 = max(0, (jb - 4 * i) * 128)
                    s_pair = ps_s.tile([128, HL, 512], F32, tag="spair")
                    for hl in range(HL):
                        h0 = hl * D
                        nc.tensor.matmul(
                            s_pair[:, hl, w0:],
                            lhsT=kT[
                                h0 : h0 + D, t0 + jb * 128 : t0 + (jb + 1) * 128
                            ],
                            rhs=qT[h0 : h0 + D, q0 + w0 : q0 + 512],
                            start=True,
                            stop=True,
                            tile_position=(h0, 0),
                        )
                    e_pair = work.tile([128, HL, 512], BF16, tag="epair", bufs=6)
                    nc.scalar.activation(
                        e_pair[:, :, w0:], s_pair[:, :, w0:], Exp, scale=0.125
                    )
                    if jb >= 4 * i:
                        nc.gpsimd.tensor_mul(
                            e_pair[:, :, w0 : w0 + 128],
                            e_pair[:, :, w0 : w0 + 128],
                            masks_sb,
                        )
                    return e_pair, w0

                def attv(jb, e_pair, w0, start, stop):
                    # va col 0 is ones -> o_ps row 0 = exp row-sums; cols
                    # 1..D-1 are zero; v channels land on rows D..2D-1
                    for hl in range(HL):
                        nc.tensor.matmul(
                            o_ps[hl][:, w0:],
                            lhsT=va_tiles[hl][:, jb, :],
                            rhs=e_pair[:, hl, w0:],
                            start=start,
                            stop=stop,
                        )

                OFF = 2
                pend = []
                emitted = 0
                for jb in range(njb):
                    pend.append((jb, scores(jb)))
                    if len(pend) > OFF:
                        pj, (pe_, pw) = pend.pop(0)
                        attv(pj, pe_, pw, start=(emitted == 0),
                             stop=(emitted == njb - 1))
                        emitted += 1
                for pj, (pe_, pw) in pend:
                    attv(pj, pe_, pw, start=(emitted == 0),
                         stop=(emitted == njb - 1))
                    emitted += 1

                # normalize: 1/rowsum from the ones column, replicated over
                # partitions by an SBUF->SBUF broadcast DMA, scale into attoT
                # normalize: copy channels to attoT unnormalized (frees the
                # PSUM bank fast), recip of the sums row on DVE, broadcast it
                # across partitions with an SWDGE DMA, then one in-place Pool
                # mul covering both heads — no PE or ACT work at all
                recips = [
                    work.tile([1, 1, 512], F32, tag=f"recips{hl}", name=f"recips{hl}")
                    for hl in range(HL)
                ]
                rb_sb = work.tile([128, 512], F32, tag="rb")
                for hl in range(HL):
                    h0 = hl * D
                    nc.vector.tensor_copy(
                        attoT[h0 : h0 + D, q0 : q0 + 512], o_ps[hl][D : 2 * D, :]
                    )
                    nc.vector.reciprocal_approx_fast(
                        recips[hl][:, 0, :], o_ps[hl][0:1, :]
                    )
                    nc.sync.dma_start(
                        rb_sb[h0 : h0 + D, :], recips[hl].to_broadcast([1, D, 512])
                    )
                flush_mul()
                pending_mul.append(
                    lambda q0=q0, rb_sb=rb_sb: nc.gpsimd.tensor_mul(
                        attoT[:, q0 : q0 + 512], attoT[:, q0 : q0 + 512], rb_sb
                    )
                )

            def c_group(tt, copy_eng):
                """Output projection for one 128-token block + bf16 store."""
                o_sb = work.tile([128, C], BF16, tag="osb", bufs=3)
                for no2 in range(2):
                    p_ps = ps_qkv.tile([128, 512], F32, tag="qkv")
                    nc.tensor.matmul(
                        p_ps,
                        lhsT=attoT[:, tt * 128 : (tt + 1) * 128],
                        rhs=wo_sb[:, no2 * 512 : (no2 + 1) * 512],
                        start=True,
                        stop=True,
                    )
                    if copy_eng == "scalar":
                        nc.scalar.copy(o_sb[:, no2 * 512 : (no2 + 1) * 512], p_ps)
                    else:
                        nc.vector.tensor_copy(
                            o_sb[:, no2 * 512 : (no2 + 1) * 512], p_ps
                        )
                nc.sync.dma_start(out[tt * 128 : (tt + 1) * 128, :], o_sb)

            # ---- pipelined emission ----
            for nt in range(NT):
                x_load(nt)
            va0 = [
                vap.tile([128, JB, 2 * D], BF16, tag="va", name=f"va0_{hl}")
                for hl in range(HL)
            ]
            va_fill(va0, 0)
            va1 = [
                vap.tile([128, JB, 2 * D], BF16, tag="va", name=f"va1_{hl}")
                for hl in range(HL)
            ]
            va_fill(va1, 1)
            _s1 = nc.enter_named_scope("W1", True)
            a_group(0)
            va_tr(va0, 0, 0)
            for i in range(QT):
                if i + 1 < QT:
                    a_group(i + 1)
                    va_tr(va0, 0, i + 1)
                a_group(QT + i)
                va_tr(va1, 1, i)
                i_tile(0, i, va0)
            nc.leave_named_scope("W1", _s1[0], True)

            _s2 = nc.enter_named_scope("W2", True)
            for i in range(QT):
                for tt in range(4 * i, 4 * i + 4):
                    c_group(tt, "vector")
                i_tile(1, i, va1)
                if i >= 1:
                    # batch-1 output projection lags its i-tile by one slot
                    for tt in range(JB + 4 * (i - 1), JB + 4 * i):
                        c_group(tt, "vector")
            nc.leave_named_scope("W2", _s2[0], True)

            _s3 = nc.enter_named_scope("W3", True)
            flush_mul()
            for tt in range(2 * JB - 4, 2 * JB):
                c_group(tt, "vector")
            nc.leave_named_scope("W3", _s3[0], True)

    _split_waits(nc)
    # populate .instr bytes for custom-DVE InstISA (reciprocal_approx_fast);
    # raw Bass skips this pass and the NEFF compiler then sees "ISA wrong
    # length"
    from concourse.library_overlay import lower_extended_insts

    lower_extended_insts(nc)
    return nc


def make_in_maps(x, Wq, bq, Wk, bk, Wv, bv, Wo, bo, with_bias):
    xT = np.ascontiguousarray(x.reshape(TOK, C).T).astype(NPBF16)
    # x4[p, nt, a, m] = x.T[a*128 + p, nt*512 + m]
    x4 = np.ascontiguousarray(
        xT.reshape(KT, 128, NT, 512).transpose(1, 2, 0, 3)
    )
    # single causal triangle [128, HL, 128]: mask[p, :, c] = 1 if c >= p
    a = np.arange(128)[:, None]
    c = np.arange(128)[None, :]
    masks = np.ascontiguousarray(
        np.repeat((c >= a).astype(NPBF16)[:, None, :], HL, axis=1)
    )
    onesz = np.zeros((128, JB, D), NPBF16)
    onesz[:, :, 0] = 1.0
    in_maps = []
    for core in range(NCORES):
        sl = slice(core * HC, (core + 1) * HC)
        def warr(W):
            # [128, KT, 128]: w3[p, a, m] = W.T[a*128 + p, m]
            return np.ascontiguousarray(
                W[sl, :].T.astype(NPBF16).reshape(KT, 128, HC).transpose(1, 0, 2)
            )

        m = {
            "x4": x4,
            "wq": warr(Wq),
            "wk": warr(Wk),
            "wv": warr(Wv),
            "wo": np.ascontiguousarray(Wo[:, sl].T).astype(NPBF16),
            "masks": masks,
            "onesz": onesz,
        }
        if with_bias:
            m["bq"] = np.ascontiguousarray(bq[sl]).reshape(HC, 1).astype(np.float32)
            m["bk"] = np.ascontiguousarray(bk[sl]).reshape(HC, 1).astype(np.float32)
            m["bv"] = np.ascontiguousarray(bv[sl]).reshape(HC, 1).astype(np.float32)
        in_maps.append(m)
    return in_maps


_NC_CACHE = {}


def kernel(x, Wq, bq, Wk, bk, Wv, bv, Wo, bo):
    x = np.asarray(x, np.float32)
    bq = np.asarray(bq, np.float32)
    bk = np.asarray(bk, np.float32)
    bv = np.asarray(bv, np.float32)
    with_bias = bool(np.any(bq) or np.any(bk) or np.any(bv))
    in_maps = make_in_maps(
        x,
        np.asarray(Wq, np.float32),
        bq,
        np.asarray(Wk, np.float32),
        bk,
        np.asarray(Wv, np.float32),
        bv,
        np.asarray(Wo, np.float32),
        np.asarray(bo, np.float32),
        with_bias,
    )
    if with_bias not in _NC_CACHE:
        _NC_CACHE[with_bias] = build(with_bias)
    trace = bool(int(os.environ.get("KERNEL_TRACE", "0")))
    res = run_bass_kernel_spmd(
        _NC_CACHE[with_bias], in_maps, core_ids=list(range(NCORES)), trace=trace
    )
    if trace:
        kernel.last_results = res
    total = np.zeros((TOK, C), np.float32)
    for core in range(NCORES):
        total += res.results[core]["out"].astype(np.float32)
    total += np.asarray(bo, np.float32)[None, :]
    return total.reshape(B, T, C)


# revision 34
# speedup vs baseline: 1.1084x; 1.0581x over previous
"""Multi-head causal attention (B=2, T=2048, C=1024, H=16) on 8 trn2 cores.

Sharding: tensor-parallel over heads. Each core computes 2 heads' QKV
projections + attention + a partial output projection; the host sums the
8 partial projections and adds the output bias.

v2: pipelined emission (QKV-projection groups interleaved with attention
i-tiles so the PE never drains), per-i-tile softmax normalization via
reciprocal_approx_fast + a K=2 broadcast matmul (replaces the serial
[1,2048] DVE reciprocal that idled the PE past the HAM window), 2-head
score matmuls packed into one PE slot via row tiling, exp merged over
both heads' PSUM banks, mask-muls on the idle GpSimd engine, bf16
partial outputs.
"""

import contextlib
import os

import ml_dtypes
import numpy as np

import bass_rust
import concourse.bass as bass
import concourse.mybir as mybir
import concourse.tile as tile
from concourse.bass_utils import run_bass_kernel_spmd

F32 = mybir.dt.float32
F32R = mybir.dt.float32r
BF16 = mybir.dt.bfloat16
NPBF16 = ml_dtypes.bfloat16

B, T, C, H = 2, 2048, 1024, 16
D = C // H          # 64
NCORES = 8
HL = H // NCORES    # heads per core = 2
TOK = B * T         # 4096
HC = HL * D         # local head channels = 128

NT = TOK // 512     # 8 token column tiles (512) over both batches
KT = C // 128       # 8 contraction tiles for projections
QT = T // 512       # 4 q tiles per batch
JB = T // 128       # 16 j (key) blocks per batch

_MAXW = 1


def _patched_drain_and_barrier(self, tick_clock, wait_clock):
    """Stock tile tail drain carries one sem-wait per outstanding proc on a
    single TPB_CTRL drain; this walrus build allows only one sync-wait per
    ctrl instruction. Split the waits across no-op carriers."""
    nc = self.nc
    carrier = nc.sync.nop()
    wait_clock.add_sem_waits(
        carrier.ins, bass_rust.ScopedClock({None: tick_clock.global_clock})
    )
    si = carrier.ins.sync_info
    waits = list(si.on_wait) if si and si.on_wait else []
    if len(waits) > _MAXW:
        carrier.ins.sync_info = mybir.SyncInfo(
            on_wait=waits[:_MAXW], on_update=list(si.on_update or [])
        )
        for i in range(_MAXW, len(waits), _MAXW):
            nop = nc.sync.nop()
            nop.ins.sync_info = mybir.SyncInfo(
                on_wait=waits[i : i + _MAXW], on_update=[]
            )
    nc.sync.drain()

    nc.all_engine_barrier()
    popped = nc._tile_sem_poison_stack.pop()
    assert popped is self._sem_poison
    assert self.sems is not None
    nc.clear_and_free_semaphores(list(self.sems.allocated().values()))
    nc.all_engine_barrier()


tile.TileContext._drain_and_barrier = _patched_drain_and_barrier


def _split_waits(nc, maxw=_MAXW):
    """This walrus build accepts at most one sync-wait per instruction.
    Hoist excess waits onto no-op carriers inserted just before the
    instruction on the same engine."""
    for f in nc.m.functions:
        for bb in f.blocks:
            insts = bb.instructions
            if not any(
                i.sync_info and i.sync_info.on_wait and len(i.sync_info.on_wait) > maxw
                for i in insts
            ):
                continue
            new = []
            for inst in insts:
                si = inst.sync_info
                waits = list(si.on_wait) if si and si.on_wait else []
                if len(waits) > maxw:
                    keep = waits[-maxw:]
                    extra = waits[:-maxw]
                    for j in range(0, len(extra), maxw):
                        nop = mybir.InstNoOp(name=nc.get_next_instruction_name())
                        nop.engine = inst.engine
                        nop.sync_info = mybir.SyncInfo(
                            on_wait=extra[j : j + maxw], on_update=[]
                        )
                        nc.register_instruction(nop)
                        new.append(nop)
                    inst.sync_info = mybir.SyncInfo(
                        on_wait=keep, on_update=list(si.on_update or [])
                    )
                new.append(inst)
            bb.instructions = new


def build(with_bias):
    nc = bass.Bass()
    # x3[p, a, m] = x.T[a*128 + p, m] — pre-rearranged on host so one DMA
    # fetches a [128, 8, 512] contraction chunk
    x4 = nc.declare_dram_parameter("x4", [128, NT, KT, 512], BF16, isOutput=False)
    wq = nc.declare_dram_parameter("wq", [128, KT, 128], BF16, isOutput=False)
    wk = nc.declare_dram_parameter("wk", [128, KT, 128], BF16, isOutput=False)
    wv = nc.declare_dram_parameter("wv", [128, KT, 128], BF16, isOutput=False)
    wo = nc.declare_dram_parameter("wo", [HC, C], BF16, isOutput=False)
    if with_bias:
        bq = nc.declare_dram_parameter("bq", [HC, 1], F32, isOutput=False)
        bk = nc.declare_dram_parameter("bk", [HC, 1], F32, isOutput=False)
        bv = nc.declare_dram_parameter("bv", [1, HC], BF16, isOutput=False)
        onesr = nc.declare_dram_parameter("onesr", [1, 128], BF16, isOutput=False)
    masks = nc.declare_dram_parameter("masks", [128, HL, 128], BF16, isOutput=False)
    onesz = nc.declare_dram_parameter("onesz", [128, JB, HL, D], BF16, isOutput=False)
    out = nc.declare_dram_parameter("out", [TOK, C], BF16, isOutput=True)

    Exp = mybir.ActivationFunctionType.Exp

    with contextlib.ExitStack() as _st:
        _st.enter_context(
            nc.allow_low_precision(reason="bf16 matmuls with fp32 accumulation")
        )
        tc = _st.enter_context(tile.TileContext(nc))
        with (
            tc.tile_pool(name="consts", bufs=1) as consts,
            tc.tile_pool(name="persist", bufs=1) as persist,
            tc.tile_pool(name="work", bufs=2) as work,
            tc.tile_pool(name="vap", bufs=2) as vap,
            tc.tile_pool(name="ps_qkv", bufs=2, space="PSUM") as ps_qkv,
            tc.tile_pool(name="ps_s", bufs=2, space="PSUM") as ps_s,
            tc.tile_pool(name="ps_o", bufs=2, space="PSUM") as ps_o,
        ):
            # ---- constants into SBUF ----
            wq_sb = consts.tile([128, KT, 128], BF16, name="wq_sb")
            wk_sb = consts.tile([128, KT, 128], BF16, name="wk_sb")
            wv_sb = consts.tile([128, KT, 128], BF16, name="wv_sb")
            for w_sb, w_dr in ((wq_sb, wq), (wk_sb, wk), (wv_sb, wv)):
                nc.sync.dma_start(w_sb, w_dr[:])
            wo_sb = consts.tile([128, C], BF16, name="wo_sb")
            nc.sync.dma_start(wo_sb, wo[:])
            if with_bias:
                bq_sb = consts.tile([HC, 1], F32, name="bq_sb")
                bk_sb = consts.tile([HC, 1], F32, name="bk_sb")
                bvr_sb = consts.tile([1, HC], BF16, name="bvr_sb")
                onesr_sb = consts.tile([1, 128], BF16, name="onesr_sb")
                for b_sb, b_dr in ((bq_sb, bq), (bk_sb, bk)):
                    nc.sync.dma_start(b_sb, b_dr[:])
                nc.sync.dma_start(bvr_sb, bv[:])
                nc.sync.dma_start(onesr_sb, onesr[:])
                biases = (bq_sb, bk_sb)
            masks_sb = consts.tile([128, HL, 128], BF16, name="masks_sb")
            nc.scalar.dma_start(masks_sb, masks[:])
            onesz_sb = consts.tile([128, JB, HL, D], BF16, name="onesz_sb")
            nc.scalar.dma_start(onesz_sb, onesz[:])

            # ---- persistent activations ----
            qT = persist.tile([HC, TOK], BF16, name="qT")
            kT = persist.tile([HC, TOK], BF16, name="kT")
            attoT = persist.tile([HC, TOK], BF16, name="attoT")

            xchunks = []

            def x_load(nt):
                xchunk = work.tile(
                    [128, KT, 512], BF16, tag="xchunk", bufs=NT, name=f"xc{nt}"
                )
                # chunk 0 on the sync ring (first, ahead of weights); the rest
                # stream on the scalar HWDGE ring before any exps are queued
                if nt == 0:
                    nc.sync.dma_start(xchunk, x4[:, nt])
                else:
                    nc.scalar.dma_start(xchunk, x4[:, nt])
                xchunks.append(xchunk)

            def a_group(nt, va_tile):
                """Q/K projections (head-major) + V projection in token-major
                for one 512-token chunk."""
                c0 = nt * 512
                xchunk = xchunks[nt]
                for ti, (w_sb, dstT) in enumerate(((wq_sb, qT), (wk_sb, kT))):
                    ps = ps_qkv.tile([128, 512], F32, tag="qkv")
                    for kt in range(KT):
                        nc.tensor.matmul(
                            ps,
                            lhsT=w_sb[:, kt, :],
                            rhs=xchunk[:, kt, :],
                            start=kt == 0,
                            stop=kt == KT - 1,
                        )
                    if with_bias:
                        nc.vector.tensor_scalar_add(
                            dstT[:, c0 : c0 + 512], ps, biases[ti]
                        )
                    else:
                        nc.vector.tensor_copy(dstT[:, c0 : c0 + 512], ps)
                # v[tok, ch]: lhsT = x columns (tokens), rhs = Wv rows
                va_ps = ps_qkv.tile([128, 4, 128], F32, tag="qkv")
                for blk in range(4):
                    for kt in range(KT):
                        nc.tensor.matmul(
                            va_ps[:, blk, :],
                            lhsT=xchunk[:, kt, blk * 128 : (blk + 1) * 128],
                            rhs=wv_sb[:, kt, :],
                            start=kt == 0,
                            stop=(kt == KT - 1 and not with_bias),
                        )
                    if with_bias:
                        nc.tensor.matmul(
                            va_ps[:, blk, :],
                            lhsT=onesr_sb,
                            rhs=bvr_sb,
                            start=False,
                            stop=True,
                        )
                for blk in range(4):
                    jb = 4 * (nt % QT) + blk
                    nc.vector.tensor_copy(
                        va_tile[:, jb, :, D : 2 * D],
                        va_ps[:, blk].rearrange("p (h c) -> p h c", h=HL),
                    )

            def va_fill(va_tile):
                """Cols [h, 0] = ones (sums row), [h, 1:D] = zeros."""
                nc.sync.dma_start(va_tile[:, :, :, 0:D], onesz_sb[:])

            pending_mul = []

            def flush_mul():
                while pending_mul:
                    pending_mul.pop(0)()

            def i_tile(b, i, va_tiles):
                """Attention for one 512-query tile, both local heads packed."""
                t0 = b * T
                q0 = t0 + i * 512
                njb = 4 * (i + 1)
                o_ps = [
                    ps_o.tile([128, 512], F32, tag="o", name=f"o{hl}")
                    for hl in range(HL)
                ]

                def scores(jb):
                    # diagonal block jb=4i+r: columns < 128r are fully masked
                    # and never computed or read; only the leading 128-wide
                    # sub-block needs the causal triangle
                    w0 = max(0, (jb - 4 * i) * 128)
                    s_pair = ps_s.tile([128, HL, 512], F32, tag="spair")
                    for hl in range(HL):
                        h0 = hl * D
                        nc.tensor.matmul(
                            s_pair[:, hl, w0:],
                            lhsT=kT[
                                h0 : h0 + D, t0 + jb * 128 : t0 + (jb + 1) * 128
                            ],
                            rhs=qT[h0 : h0 + D, q0 + w0 : q0 + 512],
                            start=True,
                            stop=True,
                            tile_position=(h0, 0),
                        )
                    e_pair = work.tile([128, HL, 512], BF16, tag="epair", bufs=6)
                    nc.scalar.activation(
                        e_pair[:, :, w0:], s_pair[:, :, w0:], Exp, scale=0.125
                    )
                    if jb >= 4 * i:
                        nc.gpsimd.tensor_mul(
                            e_pair[:, :, w0 : w0 + 128],
                            e_pair[:, :, w0 : w0 + 128],
                            masks_sb,
                        )
                    return e_pair, w0

                def attv(jb, e_pair, w0, start, stop):
                    # va col [h,0] is ones -> o_ps row 0 = exp row-sums; cols
                    # [h,1:D] zero; v channels land on rows D..2D-1
                    for hl in range(HL):
                        nc.tensor.matmul(
                            o_ps[hl][:, w0:],
                            lhsT=va_tiles[:, jb, hl, :],
                            rhs=e_pair[:, hl, w0:],
                            start=start,
                            stop=stop,
                        )

                OFF = 2
                pend = []
                emitted = 0
                for jb in range(njb):
                    pend.append((jb, scores(jb)))
                    if len(pend) > OFF:
                        pj, (pe_, pw) = pend.pop(0)
                        attv(pj, pe_, pw, start=(emitted == 0),
                             stop=(emitted == njb - 1))
                        emitted += 1
                for pj, (pe_, pw) in pend:
                    attv(pj, pe_, pw, start=(emitted == 0),
                         stop=(emitted == njb - 1))
                    emitted += 1

                # normalize: 1/rowsum from the ones column, replicated over
                # partitions by an SBUF->SBUF broadcast DMA, scale into attoT
                # normalize: copy channels to attoT unnormalized (frees the
                # PSUM bank fast), recip of the sums row on DVE, broadcast it
                # across partitions with an SWDGE DMA, then one in-place Pool
                # mul covering both heads — no PE or ACT work at all
                recips = [
                    work.tile([1, 1, 512], F32, tag=f"recips{hl}", name=f"recips{hl}")
                    for hl in range(HL)
                ]
                rb_sb = work.tile([128, 512], F32, tag="rb")
                for hl in range(HL):
                    h0 = hl * D
                    nc.vector.tensor_copy(
                        attoT[h0 : h0 + D, q0 : q0 + 512], o_ps[hl][D : 2 * D, :]
                    )
                    nc.vector.reciprocal_approx_fast(
                        recips[hl][:, 0, :], o_ps[hl][0:1, :]
                    )
                    nc.sync.dma_start(
                        rb_sb[h0 : h0 + D, :], recips[hl].to_broadcast([1, D, 512])
                    )
                flush_mul()
                pending_mul.append(
                    lambda q0=q0, rb_sb=rb_sb: nc.gpsimd.tensor_mul(
                        attoT[:, q0 : q0 + 512], attoT[:, q0 : q0 + 512], rb_sb
                    )
                )

            def c_group(tt, copy_eng):
                """Output projection for one 128-token block + bf16 store."""
                o_sb = work.tile([128, C], BF16, tag="osb", bufs=3)
                for no2 in range(2):
                    p_ps = ps_qkv.tile([128, 512], F32, tag="qkv")
                    nc.tensor.matmul(
                        p_ps,
                        lhsT=attoT[:, tt * 128 : (tt + 1) * 128],
                        rhs=wo_sb[:, no2 * 512 : (no2 + 1) * 512],
                        start=True,
                        stop=True,
                    )
                    if copy_eng == "scalar":
                        nc.scalar.copy(o_sb[:, no2 * 512 : (no2 + 1) * 512], p_ps)
                    else:
                        nc.vector.tensor_copy(
                            o_sb[:, no2 * 512 : (no2 + 1) * 512], p_ps
                        )
                nc.sync.dma_start(out[tt * 128 : (tt + 1) * 128, :], o_sb)

            # ---- pipelined emission ----
            for nt in range(NT):
                x_load(nt)
            va0 = vap.tile([128, JB, HL, 2 * D], BF16, tag="va", name="va0")
            va_fill(va0)
            va1 = vap.tile([128, JB, HL, 2 * D], BF16, tag="va", name="va1")
            va_fill(va1)
            _s1 = nc.enter_named_scope("W1", True)
            a_group(0, va0)
            for i in range(QT):
                if i + 1 < QT:
                    a_group(i + 1, va0)
                a_group(QT + i, va1)
                i_tile(0, i, va0)
            nc.leave_named_scope("W1", _s1[0], True)

            _s2 = nc.enter_named_scope("W2", True)
            for i in range(QT):
                for tt in range(4 * i, 4 * i + 4):
                    c_group(tt, "vector")
                i_tile(1, i, va1)
                if i >= 1:
                    # batch-1 output projection lags its i-tile by one slot
                    for tt in range(JB + 4 * (i - 1), JB + 4 * i):
                        c_group(tt, "vector")
            nc.leave_named_scope("W2", _s2[0], True)

            _s3 = nc.enter_named_scope("W3", True)
            flush_mul()
            for tt in range(2 * JB - 4, 2 * JB):
                c_group(tt, "vector")
            nc.leave_named_scope("W3", _s3[0], True)

    _split_waits(nc)
    # populate .instr bytes for custom-DVE InstISA (reciprocal_approx_fast);
    # raw Bass skips this pass and the NEFF compiler then sees "ISA wrong
    # length"
    from concourse.library_overlay import lower_extended_insts

    lower_extended_insts(nc)
    return nc


def make_in_maps(x, Wq, bq, Wk, bk, Wv, bv, Wo, bo, with_bias):
    xT = np.ascontiguousarray(x.reshape(TOK, C).T).astype(NPBF16)
    # x4[p, nt, a, m] = x.T[a*128 + p, nt*512 + m]
    x4 = np.ascontiguousarray(
        xT.reshape(KT, 128, NT, 512).transpose(1, 2, 0, 3)
    )
    # single causal triangle [128, HL, 128]: mask[p, :, c] = 1 if c >= p
    a = np.arange(128)[:, None]
    c = np.arange(128)[None, :]
    masks = np.ascontiguousarray(
        np.repeat((c >= a).astype(NPBF16)[:, None, :], HL, axis=1)
    )
    onesz = np.zeros((128, JB, HL, D), NPBF16)
    onesz[:, :, :, 0] = 1.0
    in_maps = []
    for core in range(NCORES):
        sl = slice(core * HC, (core + 1) * HC)
        def warr(W):
            # [128, KT, 128]: w3[p, a, m] = W.T[a*128 + p, m]
            return np.ascontiguousarray(
                W[sl, :].T.astype(NPBF16).reshape(KT, 128, HC).transpose(1, 0, 2)
            )

        m = {
            "x4": x4,
            "wq": warr(Wq),
            "wk": warr(Wk),
            "wv": warr(Wv),
            "wo": np.ascontiguousarray(Wo[:, sl].T).astype(NPBF16),
            "masks": masks,
            "onesz": onesz,
        }
        if with_bias:
            m["bq"] = np.ascontiguousarray(bq[sl]).reshape(HC, 1).astype(np.float32)
            m["bk"] = np.ascontiguousarray(bk[sl]).reshape(HC, 1).astype(np.float32)
            m["bv"] = np.ascontiguousarray(bv[sl]).reshape(1, HC).astype(NPBF16)
            m["onesr"] = np.ones((1, 128), NPBF16)
        in_maps.append(m)
    return in_maps


_NC_CACHE = {}


def kernel(x, Wq, bq, Wk, bk, Wv, bv, Wo, bo):
    x = np.asarray(x, np.float32)
    bq = np.asarray(bq, np.float32)
    bk = np.asarray(bk, np.float32)
    bv = np.asarray(bv, np.float32)
    with_bias = bool(np.any(bq) or np.any(bk) or np.any(bv))
    in_maps = make_in_maps(
        x,
        np.asarray(Wq, np.float32),
        bq,
        np.asarray(Wk, np.float32),
        bk,
        np.asarray(Wv, np.float32),
        bv,
        np.asarray(Wo, np.float32),
        np.asarray(bo, np.float32),
        with_bias,
    )
    if with_bias not in _NC_CACHE:
        _NC_CACHE[with_bias] = build(with_bias)
    trace = bool(int(os.environ.get("KERNEL_TRACE", "0")))
    res = run_bass_kernel_spmd(
        _NC_CACHE[with_bias], in_maps, core_ids=list(range(NCORES)), trace=trace
    )
    if trace:
        kernel.last_results = res
    total = np.zeros((TOK, C), np.float32)
    for core in range(NCORES):
        total += res.results[core]["out"].astype(np.float32)
    total += np.asarray(bo, np.float32)[None, :]
    return total.reshape(B, T, C)


# revision 35
# speedup vs baseline: 1.1141x; 1.0052x over previous
"""Multi-head causal attention (B=2, T=2048, C=1024, H=16) on 8 trn2 cores.

Sharding: tensor-parallel over heads. Each core computes 2 heads' QKV
projections + attention + a partial output projection; the host sums the
8 partial projections and adds the output bias.

v2: pipelined emission (QKV-projection groups interleaved with attention
i-tiles so the PE never drains), per-i-tile softmax normalization via
reciprocal_approx_fast + a K=2 broadcast matmul (replaces the serial
[1,2048] DVE reciprocal that idled the PE past the HAM window), 2-head
score matmuls packed into one PE slot via row tiling, exp merged over
both heads' PSUM banks, mask-muls on the idle GpSimd engine, bf16
partial outputs.
"""

import contextlib
import os

import ml_dtypes
import numpy as np

import bass_rust
import concourse.bass as bass
import concourse.mybir as mybir
import concourse.tile as tile
from concourse.bass_utils import run_bass_kernel_spmd

F32 = mybir.dt.float32
F32R = mybir.dt.float32r
BF16 = mybir.dt.bfloat16
NPBF16 = ml_dtypes.bfloat16

B, T, C, H = 2, 2048, 1024, 16
D = C // H          # 64
NCORES = 8
HL = H // NCORES    # heads per core = 2
TOK = B * T         # 4096
HC = HL * D         # local head channels = 128

NT = TOK // 512     # 8 token column tiles (512) over both batches
KT = C // 128       # 8 contraction tiles for projections
QT = T // 512       # 4 q tiles per batch
JB = T // 128       # 16 j (key) blocks per batch

_MAXW = 1


def _patched_drain_and_barrier(self, tick_clock, wait_clock):
    """Stock tile tail drain carries one sem-wait per outstanding proc on a
    single TPB_CTRL drain; this walrus build allows only one sync-wait per
    ctrl instruction. Split the waits across no-op carriers."""
    nc = self.nc
    carrier = nc.sync.nop()
    wait_clock.add_sem_waits(
        carrier.ins, bass_rust.ScopedClock({None: tick_clock.global_clock})
    )
    si = carrier.ins.sync_info
    waits = list(si.on_wait) if si and si.on_wait else []
    if len(waits) > _MAXW:
        carrier.ins.sync_info = mybir.SyncInfo(
            on_wait=waits[:_MAXW], on_update=list(si.on_update or [])
        )
        for i in range(_MAXW, len(waits), _MAXW):
            nop = nc.sync.nop()
            nop.ins.sync_info = mybir.SyncInfo(
                on_wait=waits[i : i + _MAXW], on_update=[]
            )
    nc.sync.drain()

    nc.all_engine_barrier()
    popped = nc._tile_sem_poison_stack.pop()
    assert popped is self._sem_poison
    assert self.sems is not None
    nc.clear_and_free_semaphores(list(self.sems.allocated().values()))
    nc.all_engine_barrier()


tile.TileContext._drain_and_barrier = _patched_drain_and_barrier


def _split_waits(nc, maxw=_MAXW):
    """This walrus build accepts at most one sync-wait per instruction.
    Hoist excess waits onto no-op carriers inserted just before the
    instruction on the same engine."""
    for f in nc.m.functions:
        for bb in f.blocks:
            insts = bb.instructions
            if not any(
                i.sync_info and i.sync_info.on_wait and len(i.sync_info.on_wait) > maxw
                for i in insts
            ):
                continue
            new = []
            for inst in insts:
                si = inst.sync_info
                waits = list(si.on_wait) if si and si.on_wait else []
                if len(waits) > maxw:
                    keep = waits[-maxw:]
                    extra = waits[:-maxw]
                    for j in range(0, len(extra), maxw):
                        nop = mybir.InstNoOp(name=nc.get_next_instruction_name())
                        nop.engine = inst.engine
                        nop.sync_info = mybir.SyncInfo(
                            on_wait=extra[j : j + maxw], on_update=[]
                        )
                        nc.register_instruction(nop)
                        new.append(nop)
                    inst.sync_info = mybir.SyncInfo(
                        on_wait=keep, on_update=list(si.on_update or [])
                    )
                new.append(inst)
            bb.instructions = new


def build(with_bias):
    nc = bass.Bass()
    # x3[p, a, m] = x.T[a*128 + p, m] — pre-rearranged on host so one DMA
    # fetches a [128, 8, 512] contraction chunk
    x4 = nc.declare_dram_parameter("x4", [128, NT, KT, 512], BF16, isOutput=False)
    wq = nc.declare_dram_parameter("wq", [128, KT, 128], BF16, isOutput=False)
    wk = nc.declare_dram_parameter("wk", [128, KT, 128], BF16, isOutput=False)
    wv = nc.declare_dram_parameter("wv", [128, KT, 128], BF16, isOutput=False)
    wo = nc.declare_dram_parameter("wo", [HC, C], BF16, isOutput=False)
    if with_bias:
        bq = nc.declare_dram_parameter("bq", [HC, 1], F32, isOutput=False)
        bk = nc.declare_dram_parameter("bk", [HC, 1], F32, isOutput=False)
        bv = nc.declare_dram_parameter("bv", [1, HC], BF16, isOutput=False)
        onesr = nc.declare_dram_parameter("onesr", [1, 128], BF16, isOutput=False)
    masks = nc.declare_dram_parameter("masks", [128, HL, 128], BF16, isOutput=False)
    out = nc.declare_dram_parameter("out", [TOK, C], BF16, isOutput=True)

    Exp = mybir.ActivationFunctionType.Exp

    with contextlib.ExitStack() as _st:
        _st.enter_context(
            nc.allow_low_precision(reason="bf16 matmuls with fp32 accumulation")
        )
        tc = _st.enter_context(tile.TileContext(nc))
        with (
            tc.tile_pool(name="consts", bufs=1) as consts,
            tc.tile_pool(name="persist", bufs=1) as persist,
            tc.tile_pool(name="work", bufs=2) as work,
            tc.tile_pool(name="vap", bufs=2) as vap,
            tc.tile_pool(name="ps_qkv", bufs=2, space="PSUM") as ps_qkv,
            tc.tile_pool(name="ps_s", bufs=2, space="PSUM") as ps_s,
            tc.tile_pool(name="ps_o", bufs=2, space="PSUM") as ps_o,
        ):
            # ---- constants into SBUF ----
            wq_sb = consts.tile([128, KT, 128], BF16, name="wq_sb")
            wk_sb = consts.tile([128, KT, 128], BF16, name="wk_sb")
            wv_sb = consts.tile([128, KT, 128], BF16, name="wv_sb")
            for w_sb, w_dr in ((wq_sb, wq), (wk_sb, wk), (wv_sb, wv)):
                nc.sync.dma_start(w_sb, w_dr[:])
            wo_sb = consts.tile([128, C], BF16, name="wo_sb")
            nc.sync.dma_start(wo_sb, wo[:])
            if with_bias:
                bq_sb = consts.tile([HC, 1], F32, name="bq_sb")
                bk_sb = consts.tile([HC, 1], F32, name="bk_sb")
                bvr_sb = consts.tile([1, HC], BF16, name="bvr_sb")
                onesr_sb = consts.tile([1, 128], BF16, name="onesr_sb")
                for b_sb, b_dr in ((bq_sb, bq), (bk_sb, bk)):
                    nc.sync.dma_start(b_sb, b_dr[:])
                nc.sync.dma_start(bvr_sb, bv[:])
                nc.sync.dma_start(onesr_sb, onesr[:])
                biases = (bq_sb, bk_sb)
            masks_sb = consts.tile([128, HL, 128], BF16, name="masks_sb")
            nc.scalar.dma_start(masks_sb, masks[:])

            # ---- persistent activations ----
            qT = persist.tile([HC, TOK], BF16, name="qT")
            kT = persist.tile([HC, TOK], BF16, name="kT")
            attoT = persist.tile([HC, TOK], BF16, name="attoT")

            xchunks = []

            def x_load(nt):
                xchunk = work.tile(
                    [128, KT, 512], BF16, tag="xchunk", bufs=NT, name=f"xc{nt}"
                )
                # chunk 0 on the sync ring (first, ahead of weights); the rest
                # stream on the scalar HWDGE ring before any exps are queued
                if nt == 0:
                    nc.sync.dma_start(xchunk, x4[:, nt])
                else:
                    nc.scalar.dma_start(xchunk, x4[:, nt])
                xchunks.append(xchunk)

            def a_group(nt, va_tile):
                """Q/K projections (head-major) + V projection in token-major
                for one 512-token chunk."""
                c0 = nt * 512
                xchunk = xchunks[nt]
                for ti, (w_sb, dstT) in enumerate(((wq_sb, qT), (wk_sb, kT))):
                    ps = ps_qkv.tile([128, 512], F32, tag="qkv")
                    for kt in range(KT):
                        nc.tensor.matmul(
                            ps,
                            lhsT=w_sb[:, kt, :],
                            rhs=xchunk[:, kt, :],
                            start=kt == 0,
                            stop=kt == KT - 1,
                        )
                    if with_bias:
                        nc.vector.tensor_scalar_add(
                            dstT[:, c0 : c0 + 512], ps, biases[ti]
                        )
                    else:
                        nc.vector.tensor_copy(dstT[:, c0 : c0 + 512], ps)
                # v[tok, ch]: lhsT = x columns (tokens), rhs = Wv rows
                va_ps = ps_qkv.tile([128, 4, 128], F32, tag="qkv")
                for blk in range(4):
                    for kt in range(KT):
                        nc.tensor.matmul(
                            va_ps[:, blk, :],
                            lhsT=xchunk[:, kt, blk * 128 : (blk + 1) * 128],
                            rhs=wv_sb[:, kt, :],
                            start=kt == 0,
                            stop=(kt == KT - 1 and not with_bias),
                        )
                    if with_bias:
                        nc.tensor.matmul(
                            va_ps[:, blk, :],
                            lhsT=onesr_sb,
                            rhs=bvr_sb,
                            start=False,
                            stop=True,
                        )
                for blk in range(4):
                    jb = 4 * (nt % QT) + blk
                    nc.vector.tensor_copy(
                        va_tile[:, jb, :, D : 2 * D],
                        va_ps[:, blk].rearrange("p (h c) -> p h c", h=HL),
                    )

            def va_fill(va_tile):
                """Cols [h, 0] = ones (sums row), [h, 1:D] = zeros."""
                nc.gpsimd.memset(va_tile[:, :, :, 0:D], 0.0)
                nc.gpsimd.memset(va_tile[:, :, :, 0:1], 1.0)

            def i_tile(b, i, va_tiles):
                """Attention for one 512-query tile, both local heads packed."""
                t0 = b * T
                q0 = t0 + i * 512
                njb = 4 * (i + 1)
                o_ps = [
                    ps_o.tile([128, 512], F32, tag="o", name=f"o{hl}")
                    for hl in range(HL)
                ]

                def scores(jb):
                    # diagonal block jb=4i+r: columns < 128r are fully masked
                    # and never computed or read; only the leading 128-wide
                    # sub-block needs the causal triangle
                    w0 = max(0, (jb - 4 * i) * 128)
                    s_pair = ps_s.tile([128, HL, 512], F32, tag="spair")
                    for hl in range(HL):
                        h0 = hl * D
                        nc.tensor.matmul(
                            s_pair[:, hl, w0:],
                            lhsT=kT[
                                h0 : h0 + D, t0 + jb * 128 : t0 + (jb + 1) * 128
                            ],
                            rhs=qT[h0 : h0 + D, q0 + w0 : q0 + 512],
                            start=True,
                            stop=True,
                            tile_position=(h0, 0),
                        )
                    e_pair = work.tile([128, HL, 512], BF16, tag="epair", bufs=6)
                    nc.scalar.activation(
                        e_pair[:, :, w0:], s_pair[:, :, w0:], Exp, scale=0.125
                    )
                    if jb >= 4 * i:
                        nc.gpsimd.tensor_mul(
                            e_pair[:, :, w0 : w0 + 128],
                            e_pair[:, :, w0 : w0 + 128],
                            masks_sb,
                        )
                    return e_pair, w0

                def attv(jb, e_pair, w0, start, stop):
                    # va col [h,0] is ones -> o_ps row 0 = exp row-sums; cols
                    # [h,1:D] zero; v channels land on rows D..2D-1
                    for hl in range(HL):
                        nc.tensor.matmul(
                            o_ps[hl][:, w0:],
                            lhsT=va_tiles[:, jb, hl, :],
                            rhs=e_pair[:, hl, w0:],
                            start=start,
                            stop=stop,
                        )

                OFF = 2
                pend = []
                emitted = 0
                for jb in range(njb):
                    pend.append((jb, scores(jb)))
                    if len(pend) > OFF:
                        pj, (pe_, pw) = pend.pop(0)
                        attv(pj, pe_, pw, start=(emitted == 0),
                             stop=(emitted == njb - 1))
                        emitted += 1
                for pj, (pe_, pw) in pend:
                    attv(pj, pe_, pw, start=(emitted == 0),
                         stop=(emitted == njb - 1))
                    emitted += 1

                # normalize: 1/rowsum from the ones column, replicated over
                # partitions by an SBUF->SBUF broadcast DMA, scale into attoT
                # normalize: copy channels to attoT unnormalized (frees the
                # PSUM bank fast), recip of the sums row on DVE, broadcast it
                # across partitions with an SWDGE DMA, then one in-place Pool
                # mul covering both heads — no PE or ACT work at all
                recips = [
                    work.tile([1, 1, 512], F32, tag=f"recips{hl}", name=f"recips{hl}")
                    for hl in range(HL)
                ]
                rb_sb = work.tile([128, 512], F32, tag="rb")
                for hl in range(HL):
                    h0 = hl * D
                    nc.vector.tensor_copy(
                        attoT[h0 : h0 + D, q0 : q0 + 512], o_ps[hl][D : 2 * D, :]
                    )
                    nc.vector.reciprocal_approx_fast(
                        recips[hl][:, 0, :], o_ps[hl][0:1, :]
                    )
                    nc.sync.dma_start(
                        rb_sb[h0 : h0 + D, :], recips[hl].to_broadcast([1, D, 512])
                    )
                nc.gpsimd.tensor_mul(
                    attoT[:, q0 : q0 + 512], attoT[:, q0 : q0 + 512], rb_sb
                )

            def c_group(tt, copy_eng):
                """Output projection for one 128-token block + bf16 store."""
                o_sb = work.tile([128, C], BF16, tag="osb", bufs=3)
                for no2 in range(2):
                    p_ps = ps_qkv.tile([128, 512], F32, tag="qkv")
                    nc.tensor.matmul(
                        p_ps,
                        lhsT=attoT[:, tt * 128 : (tt + 1) * 128],
                        rhs=wo_sb[:, no2 * 512 : (no2 + 1) * 512],
                        start=True,
                        stop=True,
                    )
                    if copy_eng == "scalar":
                        nc.scalar.copy(o_sb[:, no2 * 512 : (no2 + 1) * 512], p_ps)
                    else:
                        nc.vector.tensor_copy(
                            o_sb[:, no2 * 512 : (no2 + 1) * 512], p_ps
                        )
                nc.sync.dma_start(out[tt * 128 : (tt + 1) * 128, :], o_sb)

            # ---- pipelined emission ----
            for nt in range(NT):
                x_load(nt)
            va0 = vap.tile([128, JB, HL, 2 * D], BF16, tag="va", name="va0")
            va_fill(va0)
            va1 = vap.tile([128, JB, HL, 2 * D], BF16, tag="va", name="va1")
            va_fill(va1)
            _s1 = nc.enter_named_scope("W1", True)
            a_group(0, va0)
            for i in range(QT):
                if i + 1 < QT:
                    a_group(i + 1, va0)
                a_group(QT + i, va1)
                i_tile(0, i, va0)
            nc.leave_named_scope("W1", _s1[0], True)

            _s2 = nc.enter_named_scope("W2", True)
            for i in range(QT):
                for tt in range(4 * i, 4 * i + 4):
                    c_group(tt, "vector")
                i_tile(1, i, va1)
                if i >= 1:
                    # batch-1 output projection lags its i-tile by one slot
                    for tt in range(JB + 4 * (i - 1), JB + 4 * i):
                        c_group(tt, "vector")
            nc.leave_named_scope("W2", _s2[0], True)

            _s3 = nc.enter_named_scope("W3", True)
            for tt in range(2 * JB - 4, 2 * JB):
                c_group(tt, "vector")
            nc.leave_named_scope("W3", _s3[0], True)

    _split_waits(nc)
    # populate .instr bytes for custom-DVE InstISA (reciprocal_approx_fast);
    # raw Bass skips this pass and the NEFF compiler then sees "ISA wrong
    # length"
    from concourse.library_overlay import lower_extended_insts

    lower_extended_insts(nc)
    return nc


def make_in_maps(x, Wq, bq, Wk, bk, Wv, bv, Wo, bo, with_bias):
    xT = np.ascontiguousarray(x.reshape(TOK, C).T).astype(NPBF16)
    # x4[p, nt, a, m] = x.T[a*128 + p, nt*512 + m]
    x4 = np.ascontiguousarray(
        xT.reshape(KT, 128, NT, 512).transpose(1, 2, 0, 3)
    )
    # single causal triangle [128, HL, 128]: mask[p, :, c] = 1 if c >= p
    a = np.arange(128)[:, None]
    c = np.arange(128)[None, :]
    masks = np.ascontiguousarray(
        np.repeat((c >= a).astype(NPBF16)[:, None, :], HL, axis=1)
    )

    in_maps = []
    for core in range(NCORES):
        sl = slice(core * HC, (core + 1) * HC)
        def warr(W):
            # [128, KT, 128]: w3[p, a, m] = W.T[a*128 + p, m]
            return np.ascontiguousarray(
                W[sl, :].T.astype(NPBF16).reshape(KT, 128, HC).transpose(1, 0, 2)
            )

        m = {
            "x4": x4,
            "wq": warr(Wq),
            "wk": warr(Wk),
            "wv": warr(Wv),
            "wo": np.ascontiguousarray(Wo[:, sl].T).astype(NPBF16),
            "masks": masks,
        }
        if with_bias:
            m["bq"] = np.ascontiguousarray(bq[sl]).reshape(HC, 1).astype(np.float32)
            m["bk"] = np.ascontiguousarray(bk[sl]).reshape(HC, 1).astype(np.float32)
            m["bv"] = np.ascontiguousarray(bv[sl]).reshape(1, HC).astype(NPBF16)
            m["onesr"] = np.ones((1, 128), NPBF16)
        in_maps.append(m)
    return in_maps


_NC_CACHE = {}


def kernel(x, Wq, bq, Wk, bk, Wv, bv, Wo, bo):
    x = np.asarray(x, np.float32)
    bq = np.asarray(bq, np.float32)
    bk = np.asarray(bk, np.float32)
    bv = np.asarray(bv, np.float32)
    with_bias = bool(np.any(bq) or np.any(bk) or np.any(bv))
    in_maps = make_in_maps(
        x,
        np.asarray(Wq, np.float32),
        bq,
        np.asarray(Wk, np.float32),
        bk,
        np.asarray(Wv, np.float32),
        bv,
        np.asarray(Wo, np.float32),
        np.asarray(bo, np.float32),
        with_bias,
    )
    if with_bias not in _NC_CACHE:
        _NC_CACHE[with_bias] = build(with_bias)
    trace = bool(int(os.environ.get("KERNEL_TRACE", "0")))
    res = run_bass_kernel_spmd(
        _NC_CACHE[with_bias], in_maps, core_ids=list(range(NCORES)), trace=trace
    )
    if trace:
        kernel.last_results = res
    total = np.zeros((TOK, C), np.float32)
    for core in range(NCORES):
        total += res.results[core]["out"].astype(np.float32)
    total += np.asarray(bo, np.float32)[None, :]
    return total.reshape(B, T, C)


# revision 36
# speedup vs baseline: 1.1510x; 1.0331x over previous
"""Multi-head causal attention (B=2, T=2048, C=1024, H=16) on 8 trn2 cores.

Sharding: tensor-parallel over heads. Each core computes 2 heads' QKV
projections + attention + a partial output projection; the host sums the
8 partial projections and adds the output bias.

v2: pipelined emission (QKV-projection groups interleaved with attention
i-tiles so the PE never drains), per-i-tile softmax normalization via
reciprocal_approx_fast + a K=2 broadcast matmul (replaces the serial
[1,2048] DVE reciprocal that idled the PE past the HAM window), 2-head
score matmuls packed into one PE slot via row tiling, exp merged over
both heads' PSUM banks, mask-muls on the idle GpSimd engine, bf16
partial outputs.
"""

import contextlib
import os

import ml_dtypes
import numpy as np

import bass_rust
import concourse.bass as bass
import concourse.mybir as mybir
import concourse.tile as tile
from concourse.bass_utils import run_bass_kernel_spmd

F32 = mybir.dt.float32
F32R = mybir.dt.float32r
BF16 = mybir.dt.bfloat16
NPBF16 = ml_dtypes.bfloat16

B, T, C, H = 2, 2048, 1024, 16
D = C // H          # 64
NCORES = 8
HL = H // NCORES    # heads per core = 2
TOK = B * T         # 4096
HC = HL * D         # local head channels = 128

NT = TOK // 512     # 8 token column tiles (512) over both batches
KT = C // 128       # 8 contraction tiles for projections
QT = T // 512       # 4 q tiles per batch
JB = T // 128       # 16 j (key) blocks per batch

_MAXW = 1


def _patched_drain_and_barrier(self, tick_clock, wait_clock):
    """Stock tile tail drain carries one sem-wait per outstanding proc on a
    single TPB_CTRL drain; this walrus build allows only one sync-wait per
    ctrl instruction. Split the waits across no-op carriers."""
    nc = self.nc
    carrier = nc.sync.nop()
    wait_clock.add_sem_waits(
        carrier.ins, bass_rust.ScopedClock({None: tick_clock.global_clock})
    )
    si = carrier.ins.sync_info
    waits = list(si.on_wait) if si and si.on_wait else []
    if len(waits) > _MAXW:
        carrier.ins.sync_info = mybir.SyncInfo(
            on_wait=waits[:_MAXW], on_update=list(si.on_update or [])
        )
        for i in range(_MAXW, len(waits), _MAXW):
            nop = nc.sync.nop()
            nop.ins.sync_info = mybir.SyncInfo(
                on_wait=waits[i : i + _MAXW], on_update=[]
            )
    nc.sync.drain()

    nc.all_engine_barrier()
    popped = nc._tile_sem_poison_stack.pop()
    assert popped is self._sem_poison
    assert self.sems is not None
    nc.clear_and_free_semaphores(list(self.sems.allocated().values()))
    nc.all_engine_barrier()


tile.TileContext._drain_and_barrier = _patched_drain_and_barrier


def _split_waits(nc, maxw=_MAXW):
    """This walrus build accepts at most one sync-wait per instruction.
    Hoist excess waits onto no-op carriers inserted just before the
    instruction on the same engine."""
    for f in nc.m.functions:
        for bb in f.blocks:
            insts = bb.instructions
            if not any(
                i.sync_info and i.sync_info.on_wait and len(i.sync_info.on_wait) > maxw
                for i in insts
            ):
                continue
            new = []
            for inst in insts:
                si = inst.sync_info
                waits = list(si.on_wait) if si and si.on_wait else []
                if len(waits) > maxw:
                    keep = waits[-maxw:]
                    extra = waits[:-maxw]
                    for j in range(0, len(extra), maxw):
                        nop = mybir.InstNoOp(name=nc.get_next_instruction_name())
                        nop.engine = inst.engine
                        nop.sync_info = mybir.SyncInfo(
                            on_wait=extra[j : j + maxw], on_update=[]
                        )
                        nc.register_instruction(nop)
                        new.append(nop)
                    inst.sync_info = mybir.SyncInfo(
                        on_wait=keep, on_update=list(si.on_update or [])
                    )
                new.append(inst)
            bb.instructions = new


def build(with_bias):
    nc = bass.Bass()
    # x3[p, a, m] = x.T[a*128 + p, m] — pre-rearranged on host so one DMA
    # fetches a [128, 8, 512] contraction chunk
    x4 = nc.declare_dram_parameter("x4", [128, NT, KT, 512], BF16, isOutput=False)
    wq = nc.declare_dram_parameter("wq", [128, KT, 128], BF16, isOutput=False)
    wk = nc.declare_dram_parameter("wk", [128, KT, 128], BF16, isOutput=False)
    wv = nc.declare_dram_parameter("wv", [128, KT, 128], BF16, isOutput=False)
    wo = nc.declare_dram_parameter("wo", [HC, C], BF16, isOutput=False)
    if with_bias:
        bq = nc.declare_dram_parameter("bq", [HC, 1], F32, isOutput=False)
        bk = nc.declare_dram_parameter("bk", [HC, 1], F32, isOutput=False)
        bv = nc.declare_dram_parameter("bv", [1, HC], BF16, isOutput=False)
        onesr = nc.declare_dram_parameter("onesr", [1, 128], BF16, isOutput=False)
    masks = nc.declare_dram_parameter("masks", [128, HL, 128], BF16, isOutput=False)
    out = nc.declare_dram_parameter("out", [TOK, C], BF16, isOutput=True)

    Exp = mybir.ActivationFunctionType.Exp

    with contextlib.ExitStack() as _st:
        _st.enter_context(
            nc.allow_low_precision(reason="bf16 matmuls with fp32 accumulation")
        )
        tc = _st.enter_context(tile.TileContext(nc))
        with (
            tc.tile_pool(name="consts", bufs=1) as consts,
            tc.tile_pool(name="persist", bufs=1) as persist,
            tc.tile_pool(name="work", bufs=2) as work,
            tc.tile_pool(name="vap", bufs=2) as vap,
            tc.tile_pool(name="ps_qkv", bufs=2, space="PSUM") as ps_qkv,
            tc.tile_pool(name="ps_s", bufs=2, space="PSUM") as ps_s,
            tc.tile_pool(name="ps_o", bufs=2, space="PSUM") as ps_o,
        ):
            # ---- constants into SBUF ----
            wq_sb = consts.tile([128, KT, 128], BF16, name="wq_sb")
            wk_sb = consts.tile([128, KT, 128], BF16, name="wk_sb")
            wv_sb = consts.tile([128, KT, 128], BF16, name="wv_sb")
            for w_sb, w_dr in ((wq_sb, wq), (wk_sb, wk), (wv_sb, wv)):
                nc.sync.dma_start(w_sb, w_dr[:])
            wo_sb = consts.tile([128, C], BF16, name="wo_sb")
            nc.sync.dma_start(wo_sb, wo[:])
            if with_bias:
                bq_sb = consts.tile([HC, 1], F32, name="bq_sb")
                bk_sb = consts.tile([HC, 1], F32, name="bk_sb")
                bvr_sb = consts.tile([1, HC], BF16, name="bvr_sb")
                onesr_sb = consts.tile([1, 128], BF16, name="onesr_sb")
                for b_sb, b_dr in ((bq_sb, bq), (bk_sb, bk)):
                    nc.sync.dma_start(b_sb, b_dr[:])
                nc.sync.dma_start(bvr_sb, bv[:])
                nc.sync.dma_start(onesr_sb, onesr[:])
                biases = (bq_sb, bk_sb)
            masks_sb = consts.tile([128, HL, 128], BF16, name="masks_sb")
            nc.scalar.dma_start(masks_sb, masks[:])

            # ---- persistent activations ----
            qT = persist.tile([HC, TOK], BF16, name="qT")
            kT = persist.tile([HC, TOK], BF16, name="kT")
            attoT = persist.tile([HC, TOK], BF16, name="attoT")

            xchunks = {}

            def x_load(nt):
                xchunk = work.tile(
                    [128, KT, 512], BF16, tag="xchunk", bufs=NT, name=f"xc{nt}"
                )
                # first chunk on the sync ring right after the weights; the
                # rest trickle in on the scalar HWDGE ring with a 3-group
                # lead so the loads never jam HBM ahead of compute
                if nt == 0:
                    nc.sync.dma_start(xchunk, x4[:, nt])
                else:
                    nc.scalar.dma_start(xchunk, x4[:, nt])
                xchunks[nt] = xchunk

            def a_group(nt, va_tile):
                """Q/K projections (head-major) + V projection in token-major
                for one 512-token chunk."""
                c0 = nt * 512
                xchunk = xchunks[nt]
                for ti, (w_sb, dstT) in enumerate(((wq_sb, qT), (wk_sb, kT))):
                    ps = ps_qkv.tile([128, 512], F32, tag="qkv")
                    for kt in range(KT):
                        nc.tensor.matmul(
                            ps,
                            lhsT=w_sb[:, kt, :],
                            rhs=xchunk[:, kt, :],
                            start=kt == 0,
                            stop=kt == KT - 1,
                        )
                    if with_bias:
                        nc.vector.tensor_scalar_add(
                            dstT[:, c0 : c0 + 512], ps, biases[ti]
                        )
                    else:
                        nc.vector.tensor_copy(dstT[:, c0 : c0 + 512], ps)
                # v[tok, ch]: lhsT = x columns (tokens), rhs = Wv rows
                va_ps = ps_qkv.tile([128, 4, 128], F32, tag="qkv")
                for blk in range(4):
                    for kt in range(KT):
                        nc.tensor.matmul(
                            va_ps[:, blk, :],
                            lhsT=xchunk[:, kt, blk * 128 : (blk + 1) * 128],
                            rhs=wv_sb[:, kt, :],
                            start=kt == 0,
                            stop=(kt == KT - 1 and not with_bias),
                        )
                    if with_bias:
                        nc.tensor.matmul(
                            va_ps[:, blk, :],
                            lhsT=onesr_sb,
                            rhs=bvr_sb,
                            start=False,
                            stop=True,
                        )
                for blk in range(4):
                    jb = 4 * (nt % QT) + blk
                    nc.vector.tensor_copy(
                        va_tile[:, jb, :, D : 2 * D],
                        va_ps[:, blk].rearrange("p (h c) -> p h c", h=HL),
                    )

            def va_fill(va_tile):
                """Cols [h, 0] = ones (sums row), [h, 1:D] = zeros."""
                nc.gpsimd.memset(va_tile[:, :, :, 0:D], 0.0)
                nc.gpsimd.memset(va_tile[:, :, :, 0:1], 1.0)

            def i_tile(b, i, va_tiles):
                """Attention for one 512-query tile, both local heads packed."""
                t0 = b * T
                q0 = t0 + i * 512
                njb = 4 * (i + 1)
                o_ps = [
                    ps_o.tile([128, 512], F32, tag="o", name=f"o{hl}")
                    for hl in range(HL)
                ]

                def scores(jb):
                    # diagonal block jb=4i+r: columns < 128r are fully masked
                    # and never computed or read; only the leading 128-wide
                    # sub-block needs the causal triangle
                    w0 = max(0, (jb - 4 * i) * 128)
                    s_pair = ps_s.tile([128, HL, 512], F32, tag="spair")
                    for hl in range(HL):
                        h0 = hl * D
                        nc.tensor.matmul(
                            s_pair[:, hl, w0:],
                            lhsT=kT[
                                h0 : h0 + D, t0 + jb * 128 : t0 + (jb + 1) * 128
                            ],
                            rhs=qT[h0 : h0 + D, q0 + w0 : q0 + 512],
                            start=True,
                            stop=True,
                            tile_position=(h0, 0),
                        )
                    e_pair = work.tile([128, HL, 512], BF16, tag="epair", bufs=6)
                    nc.scalar.activation(
                        e_pair[:, :, w0:], s_pair[:, :, w0:], Exp, scale=0.125
                    )
                    if jb >= 4 * i:
                        nc.gpsimd.tensor_mul(
                            e_pair[:, :, w0 : w0 + 128],
                            e_pair[:, :, w0 : w0 + 128],
                            masks_sb,
                        )
                    return e_pair, w0

                def attv(jb, e_pair, w0, start, stop):
                    # va col [h,0] is ones -> o_ps row 0 = exp row-sums; cols
                    # [h,1:D] zero; v channels land on rows D..2D-1
                    for hl in range(HL):
                        nc.tensor.matmul(
                            o_ps[hl][:, w0:],
                            lhsT=va_tiles[:, jb, hl, :],
                            rhs=e_pair[:, hl, w0:],
                            start=start,
                            stop=stop,
                        )

                OFF = 2
                pend = []
                emitted = 0
                for jb in range(njb):
                    pend.append((jb, scores(jb)))
                    if len(pend) > OFF:
                        pj, (pe_, pw) = pend.pop(0)
                        attv(pj, pe_, pw, start=(emitted == 0),
                             stop=(emitted == njb - 1))
                        emitted += 1
                for pj, (pe_, pw) in pend:
                    attv(pj, pe_, pw, start=(emitted == 0),
                         stop=(emitted == njb - 1))
                    emitted += 1

                # normalize: 1/rowsum from the ones column, replicated over
                # partitions by an SBUF->SBUF broadcast DMA, scale into attoT
                # normalize: copy channels to attoT unnormalized (frees the
                # PSUM bank fast), recip of the sums row on DVE, broadcast it
                # across partitions with an SWDGE DMA, then one in-place Pool
                # mul covering both heads — no PE or ACT work at all
                recips = [
                    work.tile([1, 1, 512], F32, tag=f"recips{hl}", name=f"recips{hl}")
                    for hl in range(HL)
                ]
                rb_sb = work.tile([128, 512], F32, tag="rb")
                for hl in range(HL):
                    h0 = hl * D
                    nc.vector.tensor_copy(
                        attoT[h0 : h0 + D, q0 : q0 + 512], o_ps[hl][D : 2 * D, :]
                    )
                    nc.vector.reciprocal_approx_fast(
                        recips[hl][:, 0, :], o_ps[hl][0:1, :]
                    )
                    nc.sync.dma_start(
                        rb_sb[h0 : h0 + D, :], recips[hl].to_broadcast([1, D, 512])
                    )
                nc.gpsimd.tensor_mul(
                    attoT[:, q0 : q0 + 512], attoT[:, q0 : q0 + 512], rb_sb
                )

            def c_quartet(tt0, copy_eng):
                """Output projection for four 128-token blocks + one store."""
                o_sb = work.tile([128, 4, C], BF16, tag="osb", bufs=2)
                for k in range(4):
                    tt = tt0 + k
                    for no2 in range(2):
                        p_ps = ps_qkv.tile([128, 512], F32, tag="qkv")
                        nc.tensor.matmul(
                            p_ps,
                            lhsT=attoT[:, tt * 128 : (tt + 1) * 128],
                            rhs=wo_sb[:, no2 * 512 : (no2 + 1) * 512],
                            start=True,
                            stop=True,
                        )
                        if copy_eng == "scalar":
                            nc.scalar.copy(
                                o_sb[:, k, no2 * 512 : (no2 + 1) * 512], p_ps
                            )
                        else:
                            nc.vector.tensor_copy(
                                o_sb[:, k, no2 * 512 : (no2 + 1) * 512], p_ps
                            )
                nc.sync.dma_start(
                    out[tt0 * 128 : (tt0 + 4) * 128, :].rearrange(
                        "(a p) f -> p a f", p=128
                    ),
                    o_sb,
                )

            # ---- pipelined emission ----
            # W1 consumes chunks in order 0,1,4,2,5,3,6,7
            xorder = [0, 1, QT]
            for i in range(QT):
                if i + 1 < QT:
                    xorder.append(i + 2) if i + 2 < QT else None
                xorder.append(QT + i + 1) if QT + i + 1 < NT else None
            xorder = [0, 1, QT, 2, QT + 1, 3, QT + 2, QT + 3]
            for nt in xorder[:3]:
                x_load(nt)
            xq = xorder[3:]
            va0 = vap.tile([128, JB, HL, 2 * D], BF16, tag="va", name="va0")
            va_fill(va0)
            va1 = vap.tile([128, JB, HL, 2 * D], BF16, tag="va", name="va1")
            va_fill(va1)
            _s1 = nc.enter_named_scope("W1", True)
            a_group(0, va0)
            for i in range(QT):
                if xq:
                    x_load(xq.pop(0))
                if i + 1 < QT:
                    a_group(i + 1, va0)
                if xq:
                    x_load(xq.pop(0))
                a_group(QT + i, va1)
                i_tile(0, i, va0)
            nc.leave_named_scope("W1", _s1[0], True)

            _s2 = nc.enter_named_scope("W2", True)
            for i in range(QT):
                c_quartet(4 * i, "vector")
                i_tile(1, i, va1)
                if i >= 1:
                    # batch-1 output projection lags its i-tile by one slot
                    c_quartet(JB + 4 * (i - 1), "vector")
            nc.leave_named_scope("W2", _s2[0], True)

            _s3 = nc.enter_named_scope("W3", True)
            c_quartet(2 * JB - 4, "vector")
            nc.leave_named_scope("W3", _s3[0], True)

    _split_waits(nc)
    # populate .instr bytes for custom-DVE InstISA (reciprocal_approx_fast);
    # raw Bass skips this pass and the NEFF compiler then sees "ISA wrong
    # length"
    from concourse.library_overlay import lower_extended_insts

    lower_extended_insts(nc)
    return nc


def make_in_maps(x, Wq, bq, Wk, bk, Wv, bv, Wo, bo, with_bias):
    xT = np.ascontiguousarray(x.reshape(TOK, C).T).astype(NPBF16)
    # x4[p, nt, a, m] = x.T[a*128 + p, nt*512 + m]
    x4 = np.ascontiguousarray(
        xT.reshape(KT, 128, NT, 512).transpose(1, 2, 0, 3)
    )
    # single causal triangle [128, HL, 128]: mask[p, :, c] = 1 if c >= p
    a = np.arange(128)[:, None]
    c = np.arange(128)[None, :]
    masks = np.ascontiguousarray(
        np.repeat((c >= a).astype(NPBF16)[:, None, :], HL, axis=1)
    )

    in_maps = []
    for core in range(NCORES):
        sl = slice(core * HC, (core + 1) * HC)
        def warr(W):
            # [128, KT, 128]: w3[p, a, m] = W.T[a*128 + p, m]
            return np.ascontiguousarray(
                W[sl, :].T.astype(NPBF16).reshape(KT, 128, HC).transpose(1, 0, 2)
            )

        m = {
            "x4": x4,
            "wq": warr(Wq),
            "wk": warr(Wk),
            "wv": warr(Wv),
            "wo": np.ascontiguousarray(Wo[:, sl].T).astype(NPBF16),
            "masks": masks,
        }
        if with_bias:
            m["bq"] = np.ascontiguousarray(bq[sl]).reshape(HC, 1).astype(np.float32)
            m["bk"] = np.ascontiguousarray(bk[sl]).reshape(HC, 1).astype(np.float32)
            m["bv"] = np.ascontiguousarray(bv[sl]).reshape(1, HC).astype(NPBF16)
            m["onesr"] = np.ones((1, 128), NPBF16)
        in_maps.append(m)
    return in_maps


_NC_CACHE = {}


def kernel(x, Wq, bq, Wk, bk, Wv, bv, Wo, bo):
    x = np.asarray(x, np.float32)
    bq = np.asarray(bq, np.float32)
    bk = np.asarray(bk, np.float32)
    bv = np.asarray(bv, np.float32)
    with_bias = bool(np.any(bq) or np.any(bk) or np.any(bv))
    in_maps = make_in_maps(
        x,
        np.asarray(Wq, np.float32),
        bq,
        np.asarray(Wk, np.float32),
        bk,
        np.asarray(Wv, np.float32),
        bv,
        np.asarray(Wo, np.float32),
        np.asarray(bo, np.float32),
        with_bias,
    )
    if with_bias not in _NC_CACHE:
        _NC_CACHE[with_bias] = build(with_bias)
    trace = bool(int(os.environ.get("KERNEL_TRACE", "0")))
    res = run_bass_kernel_spmd(
        _NC_CACHE[with_bias], in_maps, core_ids=list(range(NCORES)), trace=trace
    )
    if trace:
        kernel.last_results = res
    total = np.zeros((TOK, C), np.float32)
    for core in range(NCORES):
        total += res.results[core]["out"].astype(np.float32)
    total += np.asarray(bo, np.float32)[None, :]
    return total.reshape(B, T, C)


# revision 37
# speedup vs baseline: 1.2043x; 1.0463x over previous
"""Multi-head causal attention (B=2, T=2048, C=1024, H=16) on 8 trn2 cores.

Sharding: tensor-parallel over heads. Each core computes 2 heads' QKV
projections + attention + a partial output projection; the host sums the
8 partial projections and adds the output bias.

v2: pipelined emission (QKV-projection groups interleaved with attention
i-tiles so the PE never drains), per-i-tile softmax normalization via
reciprocal_approx_fast + a K=2 broadcast matmul (replaces the serial
[1,2048] DVE reciprocal that idled the PE past the HAM window), 2-head
score matmuls packed into one PE slot via row tiling, exp merged over
both heads' PSUM banks, mask-muls on the idle GpSimd engine, bf16
partial outputs.
"""

import contextlib
import os

import ml_dtypes
import numpy as np

import bass_rust
import concourse.bass as bass
import concourse.mybir as mybir
import concourse.tile as tile
from concourse.bass_utils import run_bass_kernel_spmd

F32 = mybir.dt.float32
F32R = mybir.dt.float32r
BF16 = mybir.dt.bfloat16
NPBF16 = ml_dtypes.bfloat16

B, T, C, H = 2, 2048, 1024, 16
D = C // H          # 64
NCORES = 8
HL = H // NCORES    # heads per core = 2
TOK = B * T         # 4096
HC = HL * D         # local head channels = 128

NT = TOK // 512     # 8 token column tiles (512) over both batches
KT = C // 128       # 8 contraction tiles for projections
QT = T // 512       # 4 q tiles per batch
JB = T // 128       # 16 j (key) blocks per batch

_MAXW = 1


def _patched_drain_and_barrier(self, tick_clock, wait_clock):
    """Stock tile tail drain carries one sem-wait per outstanding proc on a
    single TPB_CTRL drain; this walrus build allows only one sync-wait per
    ctrl instruction. Split the waits across no-op carriers."""
    nc = self.nc
    carrier = nc.sync.nop()
    wait_clock.add_sem_waits(
        carrier.ins, bass_rust.ScopedClock({None: tick_clock.global_clock})
    )
    si = carrier.ins.sync_info
    waits = list(si.on_wait) if si and si.on_wait else []
    if len(waits) > _MAXW:
        carrier.ins.sync_info = mybir.SyncInfo(
            on_wait=waits[:_MAXW], on_update=list(si.on_update or [])
        )
        for i in range(_MAXW, len(waits), _MAXW):
            nop = nc.sync.nop()
            nop.ins.sync_info = mybir.SyncInfo(
                on_wait=waits[i : i + _MAXW], on_update=[]
            )
    nc.sync.drain()

    nc.all_engine_barrier()
    popped = nc._tile_sem_poison_stack.pop()
    assert popped is self._sem_poison
    assert self.sems is not None
    nc.clear_and_free_semaphores(list(self.sems.allocated().values()))
    nc.all_engine_barrier()


tile.TileContext._drain_and_barrier = _patched_drain_and_barrier


def _split_waits(nc, maxw=_MAXW):
    """This walrus build accepts at most one sync-wait per instruction.
    Hoist excess waits onto no-op carriers inserted just before the
    instruction on the same engine."""
    for f in nc.m.functions:
        for bb in f.blocks:
            insts = bb.instructions
            if not any(
                i.sync_info and i.sync_info.on_wait and len(i.sync_info.on_wait) > maxw
                for i in insts
            ):
                continue
            new = []
            for inst in insts:
                si = inst.sync_info
                waits = list(si.on_wait) if si and si.on_wait else []
                if len(waits) > maxw:
                    keep = waits[-maxw:]
                    extra = waits[:-maxw]
                    for j in range(0, len(extra), maxw):
                        nop = mybir.InstNoOp(name=nc.get_next_instruction_name())
                        nop.engine = inst.engine
                        nop.sync_info = mybir.SyncInfo(
                            on_wait=extra[j : j + maxw], on_update=[]
                        )
                        nc.register_instruction(nop)
                        new.append(nop)
                    inst.sync_info = mybir.SyncInfo(
                        on_wait=keep, on_update=list(si.on_update or [])
                    )
                new.append(inst)
            bb.instructions = new


def build(with_bias):
    nc = bass.Bass()
    # x3[p, a, m] = x.T[a*128 + p, m] — pre-rearranged on host so one DMA
    # fetches a [128, 8, 512] contraction chunk
    x4 = nc.declare_dram_parameter("x4", [128, NT, KT, 512], BF16, isOutput=False)
    wq = nc.declare_dram_parameter("wq", [128, KT, 128], BF16, isOutput=False)
    wk = nc.declare_dram_parameter("wk", [128, KT, 128], BF16, isOutput=False)
    wv = nc.declare_dram_parameter("wv", [128, KT, 128], BF16, isOutput=False)
    wo = nc.declare_dram_parameter("wo", [HC, C], BF16, isOutput=False)
    if with_bias:
        bq = nc.declare_dram_parameter("bq", [HC, 1], F32, isOutput=False)
        bk = nc.declare_dram_parameter("bk", [HC, 1], F32, isOutput=False)
        bv = nc.declare_dram_parameter("bv", [1, HC], BF16, isOutput=False)
        onesr = nc.declare_dram_parameter("onesr", [1, 128], BF16, isOutput=False)
    masks = nc.declare_dram_parameter("masks", [128, HL, 128], BF16, isOutput=False)
    out = nc.declare_dram_parameter("out", [TOK, C], BF16, isOutput=True)

    Exp = mybir.ActivationFunctionType.Exp

    with contextlib.ExitStack() as _st:
        _st.enter_context(
            nc.allow_low_precision(reason="bf16 matmuls with fp32 accumulation")
        )
        tc = _st.enter_context(tile.TileContext(nc))
        with (
            tc.tile_pool(name="consts", bufs=1) as consts,
            tc.tile_pool(name="persist", bufs=1) as persist,
            tc.tile_pool(name="work", bufs=2) as work,
            tc.tile_pool(name="vap", bufs=2) as vap,
            tc.tile_pool(name="ps_qkv", bufs=2, space="PSUM") as ps_qkv,
            tc.tile_pool(name="ps_s", bufs=2, space="PSUM") as ps_s,
            tc.tile_pool(name="ps_o", bufs=2, space="PSUM") as ps_o,
        ):
            # ---- x chunk 0 first: the first projections gate the pipeline
            xchunks = {}
            xc0 = work.tile([128, KT, 512], BF16, tag="xchunk", bufs=NT, name="xc0")
            nc.sync.dma_start(xc0, x4[:, 0])
            xchunks[0] = xc0

            # ---- constants into SBUF ----
            wq_sb = consts.tile([128, KT, 128], BF16, name="wq_sb")
            wk_sb = consts.tile([128, KT, 128], BF16, name="wk_sb")
            wv_sb = consts.tile([128, KT, 128], BF16, name="wv_sb")
            for w_sb, w_dr in ((wq_sb, wq), (wk_sb, wk), (wv_sb, wv)):
                nc.sync.dma_start(w_sb, w_dr[:])
            wo_sb = consts.tile([128, C], BF16, name="wo_sb")
            if with_bias:
                bq_sb = consts.tile([HC, 1], F32, name="bq_sb")
                bk_sb = consts.tile([HC, 1], F32, name="bk_sb")
                bvr_sb = consts.tile([1, HC], BF16, name="bvr_sb")
                onesr_sb = consts.tile([1, 128], BF16, name="onesr_sb")
                for b_sb, b_dr in ((bq_sb, bq), (bk_sb, bk)):
                    nc.sync.dma_start(b_sb, b_dr[:])
                nc.sync.dma_start(bvr_sb, bv[:])
                nc.sync.dma_start(onesr_sb, onesr[:])
                biases = (bq_sb, bk_sb)
            masks_sb = consts.tile([128, HL, 128], BF16, name="masks_sb")
            nc.scalar.dma_start(masks_sb, masks[:])

            # ---- persistent activations ----
            qT = persist.tile([HC, TOK], BF16, name="qT")
            kT = persist.tile([HC, TOK], BF16, name="kT")
            attoT = persist.tile([HC, TOK], BF16, name="attoT")

            def x_load(nt):
                xchunk = work.tile(
                    [128, KT, 512], BF16, tag="xchunk", bufs=NT, name=f"xc{nt}"
                )
                # chunks trickle in on the scalar HWDGE ring with a lead so
                # the loads never jam HBM ahead of compute
                nc.scalar.dma_start(xchunk, x4[:, nt])
                xchunks[nt] = xchunk

            def a_group(nt, va_tile):
                """Q/K projections (head-major) + V projection in token-major
                for one 512-token chunk."""
                c0 = nt * 512
                xchunk = xchunks[nt]
                for ti, (w_sb, dstT) in enumerate(((wq_sb, qT), (wk_sb, kT))):
                    ps = ps_qkv.tile([128, 512], F32, tag="qkv")
                    for kt in range(KT):
                        nc.tensor.matmul(
                            ps,
                            lhsT=w_sb[:, kt, :],
                            rhs=xchunk[:, kt, :],
                            start=kt == 0,
                            stop=kt == KT - 1,
                        )
                    if with_bias:
                        nc.vector.tensor_scalar_add(
                            dstT[:, c0 : c0 + 512], ps, biases[ti]
                        )
                    else:
                        nc.vector.tensor_copy(dstT[:, c0 : c0 + 512], ps)
                # v[tok, ch]: lhsT = x columns (tokens), rhs = Wv rows
                va_ps = ps_qkv.tile([128, 4, 128], F32, tag="qkv")
                for blk in range(4):
                    for kt in range(KT):
                        nc.tensor.matmul(
                            va_ps[:, blk, :],
                            lhsT=xchunk[:, kt, blk * 128 : (blk + 1) * 128],
                            rhs=wv_sb[:, kt, :],
                            start=kt == 0,
                            stop=(kt == KT - 1 and not with_bias),
                        )
                    if with_bias:
                        nc.tensor.matmul(
                            va_ps[:, blk, :],
                            lhsT=onesr_sb,
                            rhs=bvr_sb,
                            start=False,
                            stop=True,
                        )
                for blk in range(4):
                    jb = 4 * (nt % QT) + blk
                    nc.vector.tensor_copy(
                        va_tile[:, jb, :, D : 2 * D],
                        va_ps[:, blk].rearrange("p (h c) -> p h c", h=HL),
                    )

            def va_fill(va_tile):
                """Cols [h, 0] = ones (sums row), [h, 1:D] = zeros."""
                nc.gpsimd.memset(va_tile[:, :, :, 0:D], 0.0)
                nc.gpsimd.memset(va_tile[:, :, :, 0:1], 1.0)

            def i_tile(b, i, va_tiles):
                """Attention for one 512-query tile, both local heads packed."""
                t0 = b * T
                q0 = t0 + i * 512
                njb = 4 * (i + 1)
                o_ps = [
                    ps_o.tile([128, 512], F32, tag="o", name=f"o{hl}")
                    for hl in range(HL)
                ]

                def scores(jb):
                    # diagonal block jb=4i+r: columns < 128r are fully masked
                    # and never computed or read; only the leading 128-wide
                    # sub-block needs the causal triangle
                    w0 = max(0, (jb - 4 * i) * 128)
                    s_pair = ps_s.tile([128, HL, 512], F32, tag="spair")
                    for hl in range(HL):
                        h0 = hl * D
                        nc.tensor.matmul(
                            s_pair[:, hl, w0:],
                            lhsT=kT[
                                h0 : h0 + D, t0 + jb * 128 : t0 + (jb + 1) * 128
                            ],
                            rhs=qT[h0 : h0 + D, q0 + w0 : q0 + 512],
                            start=True,
                            stop=True,
                            tile_position=(h0, 0),
                        )
                    e_pair = work.tile([128, HL, 512], BF16, tag="epair", bufs=6)
                    nc.scalar.activation(
                        e_pair[:, :, w0:], s_pair[:, :, w0:], Exp, scale=0.125
                    )
                    if jb >= 4 * i:
                        nc.gpsimd.tensor_mul(
                            e_pair[:, :, w0 : w0 + 128],
                            e_pair[:, :, w0 : w0 + 128],
                            masks_sb,
                        )
                    return e_pair, w0

                def attv(jb, e_pair, w0, start, stop):
                    # va col [h,0] is ones -> o_ps row 0 = exp row-sums; cols
                    # [h,1:D] zero; v channels land on rows D..2D-1
                    for hl in range(HL):
                        nc.tensor.matmul(
                            o_ps[hl][:, w0:],
                            lhsT=va_tiles[:, jb, hl, :],
                            rhs=e_pair[:, hl, w0:],
                            start=start,
                            stop=stop,
                        )

                OFF = 2
                pend = []
                emitted = 0
                for jb in range(njb):
                    pend.append((jb, scores(jb)))
                    if len(pend) > OFF:
                        pj, (pe_, pw) = pend.pop(0)
                        attv(pj, pe_, pw, start=(emitted == 0),
                             stop=(emitted == njb - 1))
                        emitted += 1
                for pj, (pe_, pw) in pend:
                    attv(pj, pe_, pw, start=(emitted == 0),
                         stop=(emitted == njb - 1))
                    emitted += 1

                # normalize: 1/rowsum from the ones column, replicated over
                # partitions by an SBUF->SBUF broadcast DMA, scale into attoT
                # normalize: copy channels to attoT unnormalized (frees the
                # PSUM bank fast), recip of the sums row on DVE, broadcast it
                # across partitions with an SWDGE DMA, then one in-place Pool
                # mul covering both heads — no PE or ACT work at all
                recips = [
                    work.tile([1, 1, 512], F32, tag=f"recips{hl}", name=f"recips{hl}")
                    for hl in range(HL)
                ]
                rb_sb = work.tile([128, 512], F32, tag="rb")
                for hl in range(HL):
                    h0 = hl * D
                    nc.vector.tensor_copy(
                        attoT[h0 : h0 + D, q0 : q0 + 512], o_ps[hl][D : 2 * D, :]
                    )
                    nc.vector.reciprocal_approx_fast(
                        recips[hl][:, 0, :], o_ps[hl][0:1, :]
                    )
                    nc.sync.dma_start(
                        rb_sb[h0 : h0 + D, :], recips[hl].to_broadcast([1, D, 512])
                    )
                nc.gpsimd.tensor_mul(
                    attoT[:, q0 : q0 + 512], attoT[:, q0 : q0 + 512], rb_sb
                )

            def c_quartet(tt0, copy_eng):
                """Output projection for four 128-token blocks + one store."""
                o_sb = work.tile([128, 4, C], BF16, tag="osb", bufs=2)
                for k in range(4):
                    tt = tt0 + k
                    for no2 in range(2):
                        p_ps = ps_qkv.tile([128, 512], F32, tag="qkv")
                        nc.tensor.matmul(
                            p_ps,
                            lhsT=attoT[:, tt * 128 : (tt + 1) * 128],
                            rhs=wo_sb[:, no2 * 512 : (no2 + 1) * 512],
                            start=True,
                            stop=True,
                        )
                        if copy_eng == "scalar":
                            nc.scalar.copy(
                                o_sb[:, k, no2 * 512 : (no2 + 1) * 512], p_ps
                            )
                        else:
                            nc.vector.tensor_copy(
                                o_sb[:, k, no2 * 512 : (no2 + 1) * 512], p_ps
                            )
                nc.sync.dma_start(
                    out[tt0 * 128 : (tt0 + 4) * 128, :].rearrange(
                        "(a p) f -> p a f", p=128
                    ),
                    o_sb,
                )

            # ---- pipelined emission ----
            # W1 consumes chunks in order 0,1,4,2,5,3,6,7; xc0 already queued
            # ahead of the weights so nothing competes with it for HBM
            xq = [1, QT, 2, QT + 1, 3, QT + 2, QT + 3]
            va0 = vap.tile([128, JB, HL, 2 * D], BF16, tag="va", name="va0")
            va_fill(va0)
            va1 = vap.tile([128, JB, HL, 2 * D], BF16, tag="va", name="va1")
            va_fill(va1)
            _s1 = nc.enter_named_scope("W1", True)
            a_group(0, va0)
            nc.sync.dma_start(wo_sb, wo[:])
            for i in range(QT):
                if xq:
                    x_load(xq.pop(0))
                if i + 1 < QT:
                    a_group(i + 1, va0)
                if xq:
                    x_load(xq.pop(0))
                a_group(QT + i, va1)
                i_tile(0, i, va0)
            nc.leave_named_scope("W1", _s1[0], True)

            _s2 = nc.enter_named_scope("W2", True)
            for i in range(QT):
                c_quartet(4 * i, "vector")
                i_tile(1, i, va1)
                if i >= 1:
                    # batch-1 output projection lags its i-tile by one slot
                    c_quartet(JB + 4 * (i - 1), "vector")
            nc.leave_named_scope("W2", _s2[0], True)

            _s3 = nc.enter_named_scope("W3", True)
            c_quartet(2 * JB - 4, "vector")
            nc.leave_named_scope("W3", _s3[0], True)

    _split_waits(nc)
    # populate .instr bytes for custom-DVE InstISA (reciprocal_approx_fast);
    # raw Bass skips this pass and the NEFF compiler then sees "ISA wrong
    # length"
    from concourse.library_overlay import lower_extended_insts

    lower_extended_insts(nc)
    return nc


def make_in_maps(x, Wq, bq, Wk, bk, Wv, bv, Wo, bo, with_bias):
    xT = np.ascontiguousarray(x.reshape(TOK, C).T).astype(NPBF16)
    # x4[p, nt, a, m] = x.T[a*128 + p, nt*512 + m]
    x4 = np.ascontiguousarray(
        xT.reshape(KT, 128, NT, 512).transpose(1, 2, 0, 3)
    )
    # single causal triangle [128, HL, 128]: mask[p, :, c] = 1 if c >= p
    a = np.arange(128)[:, None]
    c = np.arange(128)[None, :]
    masks = np.ascontiguousarray(
        np.repeat((c >= a).astype(NPBF16)[:, None, :], HL, axis=1)
    )

    in_maps = []
    for core in range(NCORES):
        sl = slice(core * HC, (core + 1) * HC)
        def warr(W):
            # [128, KT, 128]: w3[p, a, m] = W.T[a*128 + p, m]
            return np.ascontiguousarray(
                W[sl, :].T.astype(NPBF16).reshape(KT, 128, HC).transpose(1, 0, 2)
            )

        m = {
            "x4": x4,
            "wq": warr(Wq),
            "wk": warr(Wk),
            "wv": warr(Wv),
            "wo": np.ascontiguousarray(Wo[:, sl].T).astype(NPBF16),
            "masks": masks,
        }
        if with_bias:
            m["bq"] = np.ascontiguousarray(bq[sl]).reshape(HC, 1).astype(np.float32)
            m["bk"] = np.ascontiguousarray(bk[sl]).reshape(HC, 1).astype(np.float32)
            m["bv"] = np.ascontiguousarray(bv[sl]).reshape(1, HC).astype(NPBF16)
            m["onesr"] = np.ones((1, 128), NPBF16)
        in_maps.append(m)
    return in_maps


_NC_CACHE = {}


def kernel(x, Wq, bq, Wk, bk, Wv, bv, Wo, bo):
    x = np.asarray(x, np.float32)
    bq = np.asarray(bq, np.float32)
    bk = np.asarray(bk, np.float32)
    bv = np.asarray(bv, np.float32)
    with_bias = bool(np.any(bq) or np.any(bk) or np.any(bv))
    in_maps = make_in_maps(
        x,
        np.asarray(Wq, np.float32),
        bq,
        np.asarray(Wk, np.float32),
        bk,
        np.asarray(Wv, np.float32),
        bv,
        np.asarray(Wo, np.float32),
        np.asarray(bo, np.float32),
        with_bias,
    )
    if with_bias not in _NC_CACHE:
        _NC_CACHE[with_bias] = build(with_bias)
    trace = bool(int(os.environ.get("KERNEL_TRACE", "0")))
    res = run_bass_kernel_spmd(
        _NC_CACHE[with_bias], in_maps, core_ids=list(range(NCORES)), trace=trace
    )
    if trace:
        kernel.last_results = res
    total = np.zeros((TOK, C), np.float32)
    for core in range(NCORES):
        total += res.results[core]["out"].astype(np.float32)
    total += np.asarray(bo, np.float32)[None, :]
    return total.reshape(B, T, C)
